# revision 51
# baseline (speedup 1.0000x reference)
# Trainium2 Bass kernel for nn_MultiHeadAttention_87024627352037.
#
# Full module: y = LayerNorm(x_q + (softmax(mask(QK^T/sqrt(nd))) V) Wo^T + bo)
# with Q/K/V projections of x_q/x_k/x_v. Shapes: B=2, S=2048, D=1024, H=16.
#
# Sharding (8 cores): core c = (batch b=c//4, head-quad g=c%4).
# Each core projects Q/K/V for its 4 heads (dv=256) over its batch and runs
# causal attention in a fully transposed layout (scoresT = K_T^T Q_T, no
# max-subtraction -- scores are O(1); softmax denominator via a ones-column
# in the PV matmul). Projections are streamed and interleaved with the
# attention q-tiles (processed 0,2,4,6,1,3,5,7) so the PE ramps early and
# stays busy. The ACT engine runs *only* Exp (no table reloads): the
# denominator reciprocal is computed on DVE and broadcast across partitions
# with a tiny f32r matmul; LayerNorm's rsqrt uses a DVE bit-trick + Newton
# steps. PSUM->SBUF fixups ride on the idle GPSIMD engine. A per-batch
# AllToAll (groups of 4) re-shards ctx from head-sharding to row-sharding;
# each core computes output projection + residual + LayerNorm for its 512
# rows. The host only slices, transposes, and concatenates numpy arrays.
import os
import sys
import types

import numpy as np

B, S, D, H = 2, 2048, 1024, 16
ND = D // H          # 64
NC = 8               # cores
HPC = H // 4         # 4 heads per core
DVC = HPC * ND       # 256 dv per core
QT = 256             # q tile
NQT = S // QT        # 8 q tiles
KB = 128             # k block
EPS = 1e-5
SCALE = 1.0 / np.sqrt(ND)

# iteration order: even tiles first so the even-parity AllToAll can fire at
# ~44% of the attention work and overlap the odd-tile compute.
ITERS = (0, 2, 4, 6, 1, 3, 5, 7)
# K/V 128-blocks projected at each iteration (front-loaded so tile t always
# has K/V blocks 0..2t+1 available).
KV_SCHED = {0: (0, 1), 2: (2, 3, 4, 5), 4: (6, 7, 8, 9), 6: (10, 11, 12, 13),
            1: (14, 15), 3: (), 5: (), 7: ()}

_cache = {}


def _install_ntff_shim():
    # antenv.axon_hooks is absent in this image; register the NTFF profile
    # hook so trace=True can capture HW exec time (harmless if unused).
    if "antenv.axon_hooks" in sys.modules:
        return
    mod = types.ModuleType("antenv.axon_hooks")
    mod._hook = None
    mod.set_axon_ntff_profile_hook = lambda h: setattr(mod, "_hook", h)
    mod.get_axon_ntff_profile_hook = lambda: mod._hook
    sys.modules["antenv.axon_hooks"] = mod
    try:
        import antenv

        antenv.axon_hooks = mod
        from trn_agent_boot.trn_boot import _ntff_profile_via_ctypes

        mod._hook = _ntff_profile_via_ctypes("/opt/axon/libaxon_pjrt.so")
    except Exception:
        pass


def _build():
    import concourse.bass as bass
    import concourse.mybir as mybir
    import concourse.tile as tile
    from concourse import bacc

    F32 = mybir.dt.float32
    F32R = mybir.dt.float32r
    BF16 = mybir.dt.bfloat16
    FP8 = mybir.dt.float8e4
    I32 = mybir.dt.int32
    ADD = mybir.AluOpType.add
    MUL = mybir.AluOpType.mult
    SUB = mybir.AluOpType.subtract
    SHR = mybir.AluOpType.logical_shift_right
    AF = mybir.ActivationFunctionType

    nc = bacc.Bacc("TRN2", target_bir_lowering=False, debug=False, num_devices=NC)

    def din(name, shape, dt=BF16):
        return nc.dram_tensor(name, shape, dt, kind="ExternalInput").ap()

    xtq = din("xtq", [D, S])
    xtk = din("xtk", [D, S])
    xtv = din("xtv", [D, S])
    wqT = din("wqT", [D, DVC])
    wkT = din("wkT", [D, DVC])
    wvT = din("wvT", [D, DVC])
    woT = din("woT", [D, D])
    smallc = din("smallc", [128, 288], F32)   # bq2|bk2|eps|pad|bv4x64(@16)
    gam_bc = din("gam_bc", [128, D], F32)
    bet_bc = din("bet_bc", [128, D], F32)
    resid = din("resid", [512, D], F32)       # x_q rows + bo (host pre-added)
    mo_in = din("mo", [128, 2 * QT])          # maskA|maskB (bf16)
    mo8_in = din("mo8", [128, 2 * QT], mybir.dt.float8e4)
    ones_r = din("ones_r", [1, 64], F32R)
    out_d = nc.dram_tensor("out", [512, D], F32, kind="ExternalOutput").ap()

    groups = [list(range(NC))]

    with nc.allow_low_precision(reason="f32r/bf16 matmul operand chain"), tile.TileContext(
        nc
    ) as tc:
        with (
            tc.tile_pool(name="const", bufs=1) as cpool,
            tc.tile_pool(name="res", bufs=1) as rpool,
            tc.tile_pool(name="xt", bufs=6) as xtpool,
            tc.tile_pool(name="pt", bufs=3) as ptpool,
            tc.tile_pool(name="dn", bufs=2) as dnpool,
            tc.tile_pool(name="gath", bufs=1) as gathpool,
            tc.tile_pool(name="ln", bufs=2) as lnpool,
            tc.tile_pool(name="ps_s", bufs=2, space="PSUM") as pss,
            tc.tile_pool(name="ps_ctx", bufs=2, space="PSUM") as psc,
            tc.tile_pool(name="ps_m", bufs=2, space="PSUM") as psm,
            tc.tile_pool(name="dram", bufs=1, space="DRAM") as dram,
        ):
            # ---- small constants + projection weights (needed first) ----
            smallc_sb = cpool.tile([128, 288], F32)
            mo_sb = cpool.tile([128, 2 * QT], BF16)
            mo8_sb = cpool.tile([128, 2 * QT], FP8)
            ones_sb = cpool.tile([1, 64], F32R)
            wq_sb = cpool.tile([128, 8, DVC], BF16)
            wk_sb = cpool.tile([128, 8, DVC], BF16)
            wv_sb = cpool.tile([128, 8, DVC], BF16)
            # spread startup loads across engine queues so issue overlaps
            nc.sync.dma_start(wk_sb[:], wkT.rearrange("(c p) n -> p c n", p=128))
            nc.scalar.dma_start(wq_sb[:], wqT.rearrange("(c p) n -> p c n", p=128))
            nc.gpsimd.dma_start(wv_sb[:], wvT.rearrange("(c p) n -> p c n", p=128))
            nc.sync.dma_start(smallc_sb[:], smallc)
            nc.sync.dma_start(ones_sb[:], ones_r)
            nc.sync.dma_start(mo_sb[:], mo_in)
            nc.sync.dma_start(mo8_sb[:], mo8_in)
            bq_sb = smallc_sb[:, 0:2]
            bk_sb = smallc_sb[:, 2:4]
            eps_sb = smallc_sb[:, 12:13]
            nbias_sb = smallc_sb[:, 13:14]  # -2.0 exp bias
            bv_sb = smallc_sb[:, 16:16 + DVC]   # bv broadcast (no ones col)
            mAB_sb = mo_sb[:, 0:2 * QT]

            # ---- resident activation tensors ----
            QT_sb = rpool.tile([128, 2, S], BF16)   # q^T: [dd(2x128), q]
            KT_sb = rpool.tile([128, 2, S], BF16)   # k^T: [dd(2x128), kpos]
            V_sb = rpool.tile([128, S // 128, HPC * (ND + 4)], FP8)
            ctx_sb = rpool.tile([128, 2, S], BF16)  # ctx^T: [dv(2x128), q]
            # ones columns of the V slots (denominator trick), set once;
            # slots are 68 wide (16B-aligned strides for dual-fp8 ldweights):
            # 64 data cols, a ones col, 3 zero pad cols.
            nc.gpsimd.memset(V_sb[:], 0.0)
            nc.gpsimd.memset(
                V_sb[:].rearrange("p c (h x) -> p c h x", x=ND + 4)[:, :, :, ND:ND + 1],
                1.0,
            )

            # ---- heavyweight phase-3 constants: loaded later (see below) --
            wo_sb = cpool.tile([128, 8, D], BF16)
            gam_sb = cpool.tile([128, D], F32)
            bet_sb = cpool.tile([128, D], F32)

            # ---- A2A buffers ----
            # 8-slot AllToAll (groups of 4 unsupported): slot j carries the
            # rows of dest j%4 if j's batch matches ours, zeros otherwise;
            # receivers just add slot g' and g'+4.
            a2a_in = [
                dram.tile([NC, DVC, QT], BF16, name=f"a2a_in{i}") for i in range(2)
            ]
            a2a_out = [
                dram.tile([NC, DVC, QT], BF16, name=f"a2a_out{i}") for i in range(2)
            ]
            # zsel [128, 2]: col 0 = 1.0 if our batch is 0 else 0.0; col 1 =
            # the complement. The sender writes ctx*zsel0 into the batch-0
            # slot and ctx*zsel1 into the batch-1 slot -- one is the real
            # data, the other zeros -- so receivers just add the two slots
            # (no data-dependent addressing, no receive-side select).
            zsel_sb = smallc_sb[:, 4:6]

            def proj_kq(w_sb, xt_d, b_sb, o_sb, c0):
                # project 256 source columns [c0, c0+256) into o_sb (K^T/Q^T)
                xts = xtpool.tile([128, 8, 256], BF16, tag="xt")
                nc.sync.dma_start(
                    xts[:],
                    xt_d.rearrange("(c p) n -> p c n", p=128)[:, :, c0:c0 + 256],
                )
                for m in range(2):
                    ps = psm.tile([128, 512], F32, tag="m")
                    for cc in range(8):
                        nc.tensor.matmul(
                            ps[:, 0:256],
                            lhsT=w_sb[:, cc, 128 * m:128 * m + 128],
                            rhs=xts[:, cc, :],
                            start=(cc == 0),
                            stop=(cc == 7),
                        )
                    nc.vector.tensor_scalar(
                        out=o_sb[:, m, c0:c0 + 256],
                        in0=ps[:, 0:256],
                        scalar1=b_sb[:, m:m + 1],
                        scalar2=None,
                        op0=ADD,
                    )

            def proj_v(c0):
                # project V for k rows [c0, c0+256) (two 128-blocks)
                xvs = xtpool.tile([128, 8, 256], BF16, tag="xt")
                nc.sync.dma_start(
                    xvs[:],
                    xtv.rearrange("(c p) n -> p c n", p=128)[:, :, c0:c0 + 256],
                )
                for r in range(2):
                    rc = c0 // 128 + r
                    ps = psm.tile([128, 512], F32, tag="m")
                    for cc in range(8):
                        nc.tensor.matmul(
                            ps[:, 0:DVC],
                            lhsT=xvs[:, cc, 128 * r:128 * r + 128],
                            rhs=wv_sb[:, cc, :],
                            start=(cc == 0),
                            stop=(cc == 7),
                        )
                    v_slot = V_sb[:, rc, :].rearrange("p (h x) -> p h x", x=ND + 4)[
                        :, :, 0:ND
                    ]
                    nc.vector.tensor_tensor(
                        out=v_slot,
                        in0=ps[:, 0:DVC].rearrange("p (h x) -> p h x", x=ND),
                        in1=bv_sb.rearrange("p (h x) -> p h x", x=ND),
                        op=ADD,
                    )

            def emit_pv(h, grp, pt, ctxps, t):
                co = 256 * (h % 2)
                ptv = pt.rearrange("p (b q) -> p b q", q=256)
                for idx, jp in enumerate(grp):
                    nc.tensor.matmul(
                        ctxps[0:ND + 4, co:co + 256],
                        lhsT=V_sb[:, 2 * jp:2 * jp + 2, (ND + 4) * h:(ND + 4) * (h + 1)],
                        rhs=ptv[:, 2 * idx:2 * idx + 2, :],
                        start=(jp == 0),
                        stop=(jp == t),
                        perf_mode=mybir.MatmulPerfMode.DoubleRow,
                        skip_group_check=True,
                    )

            # Deferred per-tile epilogue: the denominator broadcast matmul,
            # the normalize-divides, and the ship DMAs of tile t run during
            # iteration t+1 so the PE never waits on the reciprocal chain.
            def finish_tile(pend):
                t, ctxps_pair, dn, rcp = pend
                for pi in range(2):
                    bps = psm.tile([128, 512], F32, tag="m")
                    nc.tensor.matmul(
                        bps[0:64, :],
                        lhsT=ones_sb[0:1, :],
                        rhs=dn[0:1, 512 * pi:512 * pi + 512],
                        start=True,
                        stop=True,
                    )
                    nc.scalar.activation(
                        rcp[:, 512 * pi:512 * pi + 512],
                        bps[0:64, :],
                        AF.Copy,
                    )
                for h in range(HPC):
                    po = 64 * (h % 2)
                    hc = h // 2
                    co = 256 * (h % 2)
                    nc.vector.tensor_tensor(
                        out=ctx_sb[po:po + 64, hc, QT * t:QT * t + QT],
                        in0=ctxps_pair[h // 2][0:64, co:co + 256],
                        in1=rcp[:, 512 * (h // 2) + co:512 * (h // 2) + co + 256],
                        op=MUL,
                    )
                ha, cp = t % 2, t // 2
                ship = ptpool.tile([128, 2, 2, QT], BF16, tag="ship")
                for z in range(2):
                    nc.vector.tensor_scalar(
                        out=ship[:, z, :, :],
                        in0=ctx_sb[:, :, QT * t:QT * t + QT],
                        scalar1=zsel_sb[:, z:z + 1],
                        scalar2=None,
                        op0=MUL,
                    )
                for m in range(2):
                    nc.sync.dma_start(
                        a2a_in[ha][cp, 128 * m:128 * m + 128, :],
                        ship[:, 0, m, :],
                    )
                    nc.sync.dma_start(
                        a2a_in[ha][cp + 4, 128 * m:128 * m + 128, :],
                        ship[:, 1, m, :],
                    )
                if t == 6:
                    nc.gpsimd.collective_compute(
                        "AllToAll",
                        mybir.AluOpType.bypass,
                        replica_groups=groups,
                        ins=[a2a_in[0].opt()],
                        outs=[a2a_out[0].opt()],
                    )

            # ================= main loop =================
            pending = None
            for i, t in enumerate(ITERS):
                # ---- streamed projections for this iteration ----
                blocks = KV_SCHED[t]
                for p0 in range(0, len(blocks), 2):
                    proj_kq(wk_sb, xtk, bk_sb, KT_sb, blocks[p0] * 128)
                proj_kq(wq_sb, xtq, bq_sb, QT_sb, QT * t)
                for p0 in range(0, len(blocks), 2):
                    proj_v(blocks[p0] * 128)
                if pending is not None:
                    finish_tile(pending)
                    pending = None
                if i == 4:
                    # phase-3 constants: load mid-flight, off the hot window
                    nc.sync.dma_start(
                        wo_sb[:], woT.rearrange("(c p) n -> p c n", p=128)
                    )
                    nc.sync.dma_start(gam_sb[:], gam_bc)
                    nc.sync.dma_start(bet_sb[:], bet_bc)

                # ---- attention for q-tile t ----
                ctxps_pair = []
                for _pi in range(2):
                    cpt = psc.tile([128, 512], F32, tag="c")
                    ctxps_pair.append(cpt)
                for h in range(HPC):
                    po = 64 * (h % 2)
                    hc = h // 2
                    ctxps = ctxps_pair[h // 2]
                    q_rhs = QT_sb[po:po + 64, hc, QT * t:QT * t + QT]
                    jps = list(range(t + 1))
                    grps = [tuple(jps[k:k + 2]) for k in range(0, len(jps), 2)]
                    prev = None
                    for grp in grps:
                        w = 512 * len(grp)
                        sps = pss.tile([128, 1024], F32, tag="s")
                        for idx, jp in enumerate(grp):
                            for u in range(2):
                                nc.tensor.matmul(
                                    sps[:, 256 * (2 * idx + u):256 * (2 * idx + u) + 256],
                                    lhsT=KT_sb[
                                        po:po + 64,
                                        hc,
                                        128 * (2 * jp + u):128 * (2 * jp + u) + 128,
                                    ],
                                    rhs=q_rhs,
                                    start=True,
                                    stop=True,
                                )
                        pt = ptpool.tile([128, 1024], FP8, tag="pt")
                        # bias -2 keeps exp() under fp8e4 max; it cancels
                        # in softmax (the ones-column denominator sums the
                        # same fp8 values).
                        nc.scalar.activation(
                            pt[:, 0:w], sps[:, 0:w], AF.Exp, scale=SCALE, bias=nbias_sb
                        )
                        if t in grp:
                            do = 512 * grp.index(t)
                            nc.gpsimd.tensor_tensor(
                                out=pt[:, do:do + 512],
                                in0=pt[:, do:do + 512],
                                in1=mo8_sb,
                                op=MUL,
                            )
                        if prev is not None:
                            emit_pv(h, prev[0], prev[1], ctxps, t)
                        prev = (grp, pt)
                    emit_pv(h, prev[0], prev[1], ctxps, t)

                # ---- denominator reciprocal chain (ACT/DVE, overlaps PV) ----
                dcp = dnpool.tile([1, 1024], F32, tag="dcp")
                dn0 = dnpool.tile([1, 1024], F32, tag="dn0")
                dn = dnpool.tile([1, 1024], F32R, tag="dn")
                rcp = dnpool.tile([64, 1024], F32, tag="rcp")
                for pi in range(2):
                    nc.scalar.activation(
                        dcp[0:1, 512 * pi:512 * pi + 512],
                        ctxps_pair[pi][64:65, 0:512],
                        AF.Copy,
                    )
                    nc.vector.reciprocal_approx_fast(
                        out=dn0[0:1, 512 * pi:512 * pi + 512],
                        in_=dcp[0:1, 512 * pi:512 * pi + 512],
                    )
                    nc.vector.tensor_copy(
                        dn[0:1, 512 * pi:512 * pi + 512],
                        dn0[0:1, 512 * pi:512 * pi + 512],
                    )
                pending = (t, ctxps_pair, dn, rcp)

            # tile 7's epilogue + collective #1 dispatch FIRST, so the
            # collective's flight overlaps the ha=0 output projection.
            finish_tile(pending)
            nc.gpsimd.collective_compute(
                "AllToAll",
                mybir.AluOpType.bypass,
                replica_groups=groups,
                ins=[a2a_in[1].opt()],
                outs=[a2a_out[1].opt()],
            )

            # ---- phase 3: gather + output projection + residual + LN ----
            half = 1.5
            for ha in range(2):
                gath = gathpool.tile([128, 8, QT], BF16, tag=f"gath{ha}")
                for gp in range(4):
                    for m in range(2):
                        la = lnpool.tile([128, QT], BF16, tag="la")
                        lb = lnpool.tile([128, QT], BF16, tag="lb")
                        nc.sync.dma_start(la[:], a2a_out[ha][gp, 128 * m:128 * m + 128, :])
                        nc.sync.dma_start(lb[:], a2a_out[ha][gp + 4, 128 * m:128 * m + 128, :])
                        nc.vector.tensor_tensor(
                            out=gath[:, 2 * gp + m, :], in0=la[:], in1=lb[:], op=ADD
                        )
                for rc in range(2):
                    R = 2 * ha + rc  # local 128-row chunk index
                    y_sb = lnpool.tile([128, D], F32, tag="y")
                    res_sb = lnpool.tile([128, D], F32, tag="res")
                    nc.sync.dma_start(res_sb[:], resid[128 * R:128 * R + 128, :])
                    for n in range(2):
                        ps = psm.tile([128, 512], F32, tag="m")
                        for d2 in range(8):
                            nc.tensor.matmul(
                                ps[:],
                                lhsT=gath[:, d2, 128 * rc:128 * rc + 128],
                                rhs=wo_sb[:, d2, 512 * n:512 * n + 512],
                                start=(d2 == 0),
                                stop=(d2 == 7),
                            )
                        nc.vector.tensor_tensor(
                            out=y_sb[:, 512 * n:512 * n + 512],
                            in0=ps[:],
                            in1=res_sb[:, 512 * n:512 * n + 512],
                            op=ADD,
                        )
                    # LayerNorm over D: bn_stats mean/var + DVE rsqrt bit-trick
                    st = lnpool.tile([128, 16], F32, tag="st")
                    sti = lnpool.tile([128, 2], I32, tag="sti")
                    nc.vector.bn_stats(st[:, 0:6], y_sb[:, 0:512])
                    nc.vector.bn_stats(st[:, 6:12], y_sb[:, 512:1024])
                    nc.vector.bn_aggr(st[:, 12:14], st[:, 0:12])
                    mu = st[:, 12:13]
                    # v = var + eps; y0 = bitcast(0x5f3759df - (v_int >> 1))
                    nc.vector.tensor_tensor(
                        out=st[:, 14:15], in0=st[:, 13:14], in1=eps_sb, op=ADD
                    )
                    v = st[:, 14:15]
                    nc.vector.tensor_scalar(
                        out=sti[:, 0:1], in0=v.bitcast(I32), scalar1=1,
                        scalar2=None, op0=SHR,
                    )
                    nc.vector.tensor_scalar(
                        out=sti[:, 1:2], in0=sti[:, 0:1], scalar1=-1,
                        scalar2=0x5F3759DF, op0=MUL, op1=ADD,
                    )
                    y0 = sti[:, 1:2].bitcast(F32)
                    # h2 = -0.5 v ; two Newton steps: y <- y*(1.5 + h2*y*y)
                    nc.vector.tensor_scalar(
                        out=st[:, 15:16], in0=v, scalar1=-0.5, scalar2=None, op0=MUL
                    )
                    h2 = st[:, 15:16]
                    nc.vector.tensor_tensor(out=st[:, 0:1], in0=y0, in1=y0, op=MUL)
                    nc.vector.tensor_scalar(
                        out=st[:, 1:2], in0=st[:, 0:1], scalar1=h2, scalar2=half,
                        op0=MUL, op1=ADD,
                    )
                    nc.vector.tensor_tensor(out=st[:, 2:3], in0=y0, in1=st[:, 1:2], op=MUL)
                    nc.vector.tensor_tensor(
                        out=st[:, 3:4], in0=st[:, 2:3], in1=st[:, 2:3], op=MUL
                    )
                    nc.vector.tensor_scalar(
                        out=st[:, 4:5], in0=st[:, 3:4], scalar1=h2, scalar2=half,
                        op0=MUL, op1=ADD,
                    )
                    nc.vector.tensor_tensor(out=st[:, 5:6], in0=st[:, 2:3], in1=st[:, 4:5], op=MUL)
                    rstd = st[:, 5:6]
                    # yc = (y - mu) * rstd ; out = yc*gamma + beta
                    yc = lnpool.tile([128, D], F32, tag="yc")
                    nc.vector.tensor_scalar(
                        out=yc[:], in0=y_sb[:],
                        scalar1=mu, scalar2=rstd, op0=SUB, op1=MUL,
                    )
                    nc.vector.tensor_tensor(out=yc[:], in0=yc[:], in1=gam_sb[:], op=MUL)
                    nc.vector.tensor_tensor(out=yc[:], in0=yc[:], in1=bet_sb[:], op=ADD)
                    nc.sync.dma_start(out_d[128 * R:128 * R + 128, :], yc[:])

    nc.compile()
    return nc


def _prep_inputs(x_q, x_k, x_v, mask, Wq, bq, Wk, bk, Wv, bv, Wo, bo, gamma, beta):
    import ml_dtypes

    f = np.float32
    bf = ml_dtypes.bfloat16
    maskA = np.zeros((KB, QT), f)
    maskB = np.zeros((KB, QT), f)
    for i in range(KB):
        maskA[i, i:] = 1.0
        if i + 128 < QT:
            maskB[i, i + 128:] = 1.0
    mo = np.concatenate([maskA, maskB], axis=1).astype(bf)
    mo8 = np.concatenate([maskA, maskB], axis=1).astype(ml_dtypes.float8_e4m3)
    in_maps = []
    for c in range(NC):
        b, g = c // 4, c % 4
        dv = slice(DVC * g, DVC * (g + 1))
        smallc = np.zeros((128, 288), f)
        smallc[:, 0:2] = bq[dv].astype(f).reshape(2, 128).T
        smallc[:, 2:4] = bk[dv].astype(f).reshape(2, 128).T
        smallc[:, 4] = 1.0 - b
        smallc[:, 5] = float(b)
        smallc[:, 12] = EPS
        smallc[:, 13] = -2.0
        smallc[:, 16:16 + DVC] = np.broadcast_to(bv[dv].astype(f), (128, DVC))
        in_maps.append(
            {
                "xtq": np.ascontiguousarray(x_q[b].T.astype(bf)),
                "xtk": np.ascontiguousarray(x_k[b].T.astype(bf)),
                "xtv": np.ascontiguousarray(x_v[b].T.astype(bf)),
                "wqT": np.ascontiguousarray(Wq[dv, :].T.astype(bf)),
                "wkT": np.ascontiguousarray(Wk[dv, :].T.astype(bf)),
                "wvT": np.ascontiguousarray(Wv[dv, :].T.astype(bf)),
                "woT": np.ascontiguousarray(Wo.T.astype(bf)),
                "smallc": smallc,
                "gam_bc": np.broadcast_to(gamma.astype(f), (128, D)).copy(),
                "bet_bc": np.broadcast_to(beta.astype(f), (128, D)).copy(),
                "resid": np.ascontiguousarray(
                    x_q[b, 512 * g:512 * (g + 1), :].astype(f) + bo.astype(f)
                ),
                "mo": mo,
                "mo8": mo8,
                "ones_r": np.ones((1, 64), f),
            }
        )
    return in_maps


def kernel(x_q, x_k, x_v, mask, Wq, bq, Wk, bk, Wv, bv, Wo, bo, gamma, beta):
    _install_ntff_shim()
    from concourse.bass_utils import run_bass_kernel_spmd

    x_q, x_k, x_v = np.asarray(x_q), np.asarray(x_k), np.asarray(x_v)
    mask = np.asarray(mask)
    # this kernel implements causal attention structurally; verify the mask
    causal = np.tril(np.ones((S, S), mask.dtype))
    assert np.array_equal(mask.reshape(S, S), causal), "kernel specialized for causal mask"

    if "nc" not in _cache:
        _cache["nc"] = _build()
    nc = _cache["nc"]

    in_maps = _prep_inputs(
        x_q, x_k, x_v, mask,
        np.asarray(Wq), np.asarray(bq), np.asarray(Wk), np.asarray(bk),
        np.asarray(Wv), np.asarray(bv), np.asarray(Wo), np.asarray(bo),
        np.asarray(gamma), np.asarray(beta),
    )
    res = run_bass_kernel_spmd(nc, in_maps, list(range(NC)))
    _cache["last_results"] = res

    out = np.empty((B, S, D), np.float32)
    for c in range(NC):
        b, g = c // 4, c % 4
        out[b, 512 * g:512 * (g + 1), :] = res.results[c]["out"]
    return out


# revision 52
# speedup vs baseline: 1.0914x; 1.0914x over previous
# Trainium2 Bass kernel for nn_MultiHeadAttention_87024627352037.
#
# Full module: y = LayerNorm(x_q + (softmax(mask(QK^T/sqrt(nd))) V) Wo^T + bo)
# with Q/K/V projections of x_q/x_k/x_v. Shapes: B=2, S=2048, D=1024, H=16.
#
# Sharding (8 cores): core c = (batch b=c//4, head-quad g=c%4).
# Each core projects Q/K/V for its 4 heads (dv=256) over its batch and runs
# causal attention in a fully transposed layout (scoresT = K_T^T Q_T, no
# max-subtraction -- scores are O(1); softmax denominator via a ones-column
# in the PV matmul). Projections are streamed and interleaved with the
# attention q-tiles (processed 0,2,4,6,1,3,5,7) so the PE ramps early and
# stays busy. The ACT engine runs *only* Exp (no table reloads): the
# denominator reciprocal is computed on DVE and broadcast across partitions
# with a tiny f32r matmul; LayerNorm's rsqrt uses a DVE bit-trick + Newton
# steps. PSUM->SBUF fixups ride on the idle GPSIMD engine. A per-batch
# AllToAll (groups of 4) re-shards ctx from head-sharding to row-sharding;
# each core computes output projection + residual + LayerNorm for its 512
# rows. The host only slices, transposes, and concatenates numpy arrays.
import os
import sys
import types

import numpy as np

B, S, D, H = 2, 2048, 1024, 16
ND = D // H          # 64
NC = 8               # cores
HPC = H // 4         # 4 heads per core
DVC = HPC * ND       # 256 dv per core
QT = 256             # q tile
NQT = S // QT        # 8 q tiles
KB = 128             # k block
EPS = 1e-5
SCALE = 1.0 / np.sqrt(ND)

# iteration order: even tiles first so the even-parity AllToAll can fire at
# ~44% of the attention work and overlap the odd-tile compute.
ITERS = (0, 2, 4, 6, 1, 3, 5, 7)
# K/V 128-blocks projected at each iteration (front-loaded so tile t always
# has K/V blocks 0..2t+1 available).
KV_SCHED = {0: (0, 1), 2: (2, 3, 4, 5), 4: (6, 7, 8, 9), 6: (10, 11, 12, 13),
            1: (14, 15), 3: (), 5: (), 7: ()}

_cache = {}


def _install_ntff_shim():
    # antenv.axon_hooks is absent in this image; register the NTFF profile
    # hook so trace=True can capture HW exec time (harmless if unused).
    if "antenv.axon_hooks" in sys.modules:
        return
    mod = types.ModuleType("antenv.axon_hooks")
    mod._hook = None
    mod.set_axon_ntff_profile_hook = lambda h: setattr(mod, "_hook", h)
    mod.get_axon_ntff_profile_hook = lambda: mod._hook
    sys.modules["antenv.axon_hooks"] = mod
    try:
        import antenv

        antenv.axon_hooks = mod
        from trn_agent_boot.trn_boot import _ntff_profile_via_ctypes

        mod._hook = _ntff_profile_via_ctypes("/opt/axon/libaxon_pjrt.so")
    except Exception:
        pass


def _build():
    import concourse.bass as bass
    import concourse.mybir as mybir
    import concourse.tile as tile
    from concourse import bacc

    F32 = mybir.dt.float32
    F32R = mybir.dt.float32r
    BF16 = mybir.dt.bfloat16
    FP8 = mybir.dt.float8e4
    I32 = mybir.dt.int32
    ADD = mybir.AluOpType.add
    MUL = mybir.AluOpType.mult
    SUB = mybir.AluOpType.subtract
    SHR = mybir.AluOpType.logical_shift_right
    AF = mybir.ActivationFunctionType

    nc = bacc.Bacc("TRN2", target_bir_lowering=False, debug=False, num_devices=NC)

    def din(name, shape, dt=BF16):
        return nc.dram_tensor(name, shape, dt, kind="ExternalInput").ap()

    xtq = din("xtq", [D, S])
    xtk = din("xtk", [D, S])
    xtv = din("xtv", [D, S])
    wqT = din("wqT", [D, DVC])
    wkT = din("wkT", [D, DVC])
    wvT = din("wvT", [D, DVC])
    woT = din("woT", [D, D])
    smallc = din("smallc", [128, 288], F32)   # bq2|bk2|eps|pad|bv4x64(@16)
    gam_bc = din("gam_bc", [128, D], F32)
    bet_bc = din("bet_bc", [128, D], F32)
    resid = din("resid", [512, D], F32)       # x_q rows + bo (host pre-added)
    mo_in = din("mo", [128, 2 * QT])          # maskA|maskB (bf16)
    mo8_in = din("mo8", [128, 2 * QT], mybir.dt.float8e4)
    ones_r = din("ones_r", [1, 64], F32R)
    out_d = nc.dram_tensor("out", [512, D], F32, kind="ExternalOutput").ap()

    groups = [list(range(NC))]

    with nc.allow_low_precision(reason="f32r/bf16 matmul operand chain"), tile.TileContext(
        nc
    ) as tc:
        with (
            tc.tile_pool(name="const", bufs=1) as cpool,
            tc.tile_pool(name="res", bufs=1) as rpool,
            tc.tile_pool(name="xt", bufs=6) as xtpool,
            tc.tile_pool(name="pt", bufs=3) as ptpool,
            tc.tile_pool(name="dn", bufs=2) as dnpool,
            tc.tile_pool(name="gath", bufs=1) as gathpool,
            tc.tile_pool(name="ln", bufs=2) as lnpool,
            tc.tile_pool(name="ps_s", bufs=2, space="PSUM") as pss,
            tc.tile_pool(name="ps_ctx", bufs=2, space="PSUM") as psc,
            tc.tile_pool(name="ps_m", bufs=2, space="PSUM") as psm,
            tc.tile_pool(name="dram", bufs=1, space="DRAM") as dram,
        ):
            # ---- small constants + projection weights (needed first) ----
            smallc_sb = cpool.tile([128, 288], F32)
            mo_sb = cpool.tile([128, 2 * QT], BF16)
            mo8_sb = cpool.tile([128, 2 * QT], FP8)
            ones_sb = cpool.tile([1, 64], F32R)
            wq_sb = cpool.tile([128, 8, DVC], BF16)
            wk_sb = cpool.tile([128, 8, DVC], BF16)
            wv_sb = cpool.tile([128, 8, DVC], BF16)
            # spread startup loads across engine queues so issue overlaps
            nc.sync.dma_start(wk_sb[:], wkT.rearrange("(c p) n -> p c n", p=128))
            nc.scalar.dma_start(wq_sb[:], wqT.rearrange("(c p) n -> p c n", p=128))
            nc.gpsimd.dma_start(wv_sb[:], wvT.rearrange("(c p) n -> p c n", p=128))
            nc.sync.dma_start(smallc_sb[:], smallc)
            nc.sync.dma_start(ones_sb[:], ones_r)
            nc.sync.dma_start(mo_sb[:], mo_in)
            nc.sync.dma_start(mo8_sb[:], mo8_in)
            bq_sb = smallc_sb[:, 0:2]
            bk_sb = smallc_sb[:, 2:4]
            eps_sb = smallc_sb[:, 12:13]
            nbias_sb = smallc_sb[:, 13:14]  # -2.0 exp bias
            bv_sb = smallc_sb[:, 16:16 + DVC]   # bv broadcast (no ones col)
            mAB_sb = mo_sb[:, 0:2 * QT]

            # ---- resident activation tensors ----
            QT_sb = rpool.tile([128, 2, S], BF16)   # q^T: [dd(2x128), q]
            KT_sb = rpool.tile([128, 2, S], BF16)   # k^T: [dd(2x128), kpos]
            V_sb = rpool.tile([128, S // 128, HPC * (ND + 4)], FP8)
            ctx_sb = rpool.tile([128, 2, S], BF16)  # ctx^T: [dv(2x128), q]
            # ones columns of the V slots (denominator trick), set once;
            # slots are 68 wide (16B-aligned strides for dual-fp8 ldweights):
            # 64 data cols, a ones col, 3 zero pad cols.
            nc.gpsimd.memset(V_sb[:], 0.0)
            nc.gpsimd.memset(
                V_sb[:].rearrange("p c (h x) -> p c h x", x=ND + 4)[:, :, :, ND:ND + 1],
                1.0,
            )

            # ---- heavyweight phase-3 constants: loaded later (see below) --
            wo_sb = cpool.tile([128, 8, D], BF16)
            gam_sb = cpool.tile([128, D], F32)
            bet_sb = cpool.tile([128, D], F32)

            # ---- A2A buffers ----
            # 8-slot AllToAll (groups of 4 unsupported): slot j carries the
            # rows of dest j%4 if j's batch matches ours, zeros otherwise;
            # receivers just add slot g' and g'+4.
            a2a_in = [
                dram.tile([NC, DVC, QT], BF16, name=f"a2a_in{i}") for i in range(2)
            ]
            a2a_out = [
                dram.tile([NC, DVC, QT], BF16, name=f"a2a_out{i}") for i in range(2)
            ]
            # zsel [128, 2]: col 0 = 1.0 if our batch is 0 else 0.0; col 1 =
            # the complement. The sender writes ctx*zsel0 into the batch-0
            # slot and ctx*zsel1 into the batch-1 slot -- one is the real
            # data, the other zeros -- so receivers just add the two slots
            # (no data-dependent addressing, no receive-side select).
            zsel_sb = smallc_sb[:, 4:6]

            def proj_kq(w_sb, xt_d, b_sb, o_sb, c0):
                # project 256 source columns [c0, c0+256) into o_sb (K^T/Q^T)
                xts = xtpool.tile([128, 8, 256], BF16, tag="xt")
                nc.sync.dma_start(
                    xts[:],
                    xt_d.rearrange("(c p) n -> p c n", p=128)[:, :, c0:c0 + 256],
                )
                for m in range(2):
                    ps = psm.tile([128, 512], F32, tag="m")
                    for cc in range(8):
                        nc.tensor.matmul(
                            ps[:, 0:256],
                            lhsT=w_sb[:, cc, 128 * m:128 * m + 128],
                            rhs=xts[:, cc, :],
                            start=(cc == 0),
                            stop=(cc == 7),
                        )
                    nc.vector.tensor_scalar(
                        out=o_sb[:, m, c0:c0 + 256],
                        in0=ps[:, 0:256],
                        scalar1=b_sb[:, m:m + 1],
                        scalar2=None,
                        op0=ADD,
                    )

            def proj_v(c0):
                # project V for k rows [c0, c0+256) (two 128-blocks)
                xvs = xtpool.tile([128, 8, 256], BF16, tag="xt")
                nc.sync.dma_start(
                    xvs[:],
                    xtv.rearrange("(c p) n -> p c n", p=128)[:, :, c0:c0 + 256],
                )
                for r in range(2):
                    rc = c0 // 128 + r
                    ps = psm.tile([128, 512], F32, tag="m")
                    for cc in range(8):
                        nc.tensor.matmul(
                            ps[:, 0:DVC],
                            lhsT=xvs[:, cc, 128 * r:128 * r + 128],
                            rhs=wv_sb[:, cc, :],
                            start=(cc == 0),
                            stop=(cc == 7),
                        )
                    v_slot = V_sb[:, rc, :].rearrange("p (h x) -> p h x", x=ND + 4)[
                        :, :, 0:ND
                    ]
                    nc.vector.tensor_tensor(
                        out=v_slot,
                        in0=ps[:, 0:DVC].rearrange("p (h x) -> p h x", x=ND),
                        in1=bv_sb.rearrange("p (h x) -> p h x", x=ND),
                        op=ADD,
                    )

            def emit_pv(h, grp, pt, ctxps, t):
                co = 256 * (h % 2)
                ptv = pt.rearrange("p (b q) -> p b q", q=256)
                for idx, jp in enumerate(grp):
                    nc.tensor.matmul(
                        ctxps[0:ND + 4, co:co + 256],
                        lhsT=V_sb[:, 2 * jp:2 * jp + 2, (ND + 4) * h:(ND + 4) * (h + 1)],
                        rhs=ptv[:, 2 * idx:2 * idx + 2, :],
                        start=(jp == 0),
                        stop=(jp == t),
                        perf_mode=mybir.MatmulPerfMode.DoubleRow,
                        skip_group_check=True,
                    )

            # Deferred per-tile epilogue: the denominator broadcast matmul,
            # the normalize-divides, and the ship DMAs of tile t run during
            # iteration t+1 so the PE never waits on the reciprocal chain.
            def finish_tile(pend):
                t, ctxps_pair, dn, rcp = pend
                for pi in range(2):
                    bps = psm.tile([128, 512], F32, tag="m")
                    nc.tensor.matmul(
                        bps[0:64, :],
                        lhsT=ones_sb[0:1, :],
                        rhs=dn[0:1, 512 * pi:512 * pi + 512],
                        start=True,
                        stop=True,
                    )
                    nc.vector.tensor_copy(
                        rcp[:, 512 * pi:512 * pi + 512],
                        bps[0:64, :],
                    )
                for h in range(HPC):
                    po = 64 * (h % 2)
                    hc = h // 2
                    co = 256 * (h % 2)
                    nc.vector.tensor_tensor(
                        out=ctx_sb[po:po + 64, hc, QT * t:QT * t + QT],
                        in0=ctxps_pair[h // 2][0:64, co:co + 256],
                        in1=rcp[:, 512 * (h // 2) + co:512 * (h // 2) + co + 256],
                        op=MUL,
                    )
                ha, cp = t % 2, t // 2
                ship = ptpool.tile([128, 2, 2, QT], BF16, tag="ship")
                for z in range(2):
                    nc.vector.tensor_scalar(
                        out=ship[:, z, :, :],
                        in0=ctx_sb[:, :, QT * t:QT * t + QT],
                        scalar1=zsel_sb[:, z:z + 1],
                        scalar2=None,
                        op0=MUL,
                    )
                for m in range(2):
                    nc.sync.dma_start(
                        a2a_in[ha][cp, 128 * m:128 * m + 128, :],
                        ship[:, 0, m, :],
                    )
                    nc.sync.dma_start(
                        a2a_in[ha][cp + 4, 128 * m:128 * m + 128, :],
                        ship[:, 1, m, :],
                    )
                if t == 6:
                    nc.gpsimd.collective_compute(
                        "AllToAll",
                        mybir.AluOpType.bypass,
                        replica_groups=groups,
                        ins=[a2a_in[0].opt()],
                        outs=[a2a_out[0].opt()],
                    )

            # ================= main loop =================
            pending = None
            for i, t in enumerate(ITERS):
                # ---- streamed projections for this iteration ----
                blocks = KV_SCHED[t]
                for p0 in range(0, len(blocks), 2):
                    c0 = blocks[p0] * 128
                    proj_kq(wk_sb, xtk, bk_sb, KT_sb, c0)
                    proj_v(c0)
                proj_kq(wq_sb, xtq, bq_sb, QT_sb, QT * t)
                if pending is not None:
                    finish_tile(pending)
                    pending = None
                if i == 4:
                    # phase-3 constants: load mid-flight, off the hot window
                    nc.sync.dma_start(
                        wo_sb[:], woT.rearrange("(c p) n -> p c n", p=128)
                    )
                    nc.sync.dma_start(gam_sb[:], gam_bc)
                    nc.sync.dma_start(bet_sb[:], bet_bc)

                # ---- attention for q-tile t ----
                ctxps_pair = []
                for _pi in range(2):
                    cpt = psc.tile([128, 512], F32, tag="c")
                    ctxps_pair.append(cpt)
                for h in range(HPC):
                    po = 64 * (h % 2)
                    hc = h // 2
                    ctxps = ctxps_pair[h // 2]
                    q_rhs = QT_sb[po:po + 64, hc, QT * t:QT * t + QT]
                    jps = list(range(t + 1))
                    grps = [tuple(jps[k:k + 2]) for k in range(0, len(jps), 2)]
                    prev = None
                    for grp in grps:
                        w = 512 * len(grp)
                        sps = pss.tile([128, 1024], F32, tag="s")
                        for idx, jp in enumerate(grp):
                            for u in range(2):
                                nc.tensor.matmul(
                                    sps[:, 256 * (2 * idx + u):256 * (2 * idx + u) + 256],
                                    lhsT=KT_sb[
                                        po:po + 64,
                                        hc,
                                        128 * (2 * jp + u):128 * (2 * jp + u) + 128,
                                    ],
                                    rhs=q_rhs,
                                    start=True,
                                    stop=True,
                                )
                        pt = ptpool.tile([128, 1024], FP8, tag="pt")
                        # bias -2 keeps exp() under fp8e4 max; it cancels
                        # in softmax (the ones-column denominator sums the
                        # same fp8 values).
                        nc.scalar.activation(
                            pt[:, 0:w], sps[:, 0:w], AF.Exp, scale=SCALE, bias=nbias_sb
                        )
                        if t in grp:
                            do = 512 * grp.index(t)
                            nc.vector.tensor_tensor(
                                out=pt[:, do:do + 512],
                                in0=pt[:, do:do + 512],
                                in1=mo8_sb,
                                op=MUL,
                            )
                        if prev is not None:
                            emit_pv(h, prev[0], prev[1], ctxps, t)
                        prev = (grp, pt)
                    emit_pv(h, prev[0], prev[1], ctxps, t)

                # ---- denominator reciprocal chain (ACT/DVE, overlaps PV) ----
                dcp = dnpool.tile([1, 1024], F32, tag="dcp")
                dn0 = dnpool.tile([1, 1024], F32, tag="dn0")
                dn = dnpool.tile([1, 1024], F32R, tag="dn")
                rcp = dnpool.tile([64, 1024], F32, tag="rcp")
                for pi in range(2):
                    nc.scalar.activation(
                        dcp[0:1, 512 * pi:512 * pi + 512],
                        ctxps_pair[pi][64:65, 0:512],
                        AF.Copy,
                    )
                    nc.vector.reciprocal_approx_fast(
                        out=dn0[0:1, 512 * pi:512 * pi + 512],
                        in_=dcp[0:1, 512 * pi:512 * pi + 512],
                    )
                    nc.vector.tensor_copy(
                        dn[0:1, 512 * pi:512 * pi + 512],
                        dn0[0:1, 512 * pi:512 * pi + 512],
                    )
                pending = (t, ctxps_pair, dn, rcp)

            # tile 7's epilogue + collective #1 dispatch FIRST, so the
            # collective's flight overlaps the ha=0 output projection.
            finish_tile(pending)
            nc.gpsimd.collective_compute(
                "AllToAll",
                mybir.AluOpType.bypass,
                replica_groups=groups,
                ins=[a2a_in[1].opt()],
                outs=[a2a_out[1].opt()],
            )

            # ---- phase 3: gather + output projection + residual + LN ----
            half = 1.5
            for ha in range(2):
                gath = gathpool.tile([128, 8, QT], BF16, tag=f"gath{ha}")
                for gp in range(4):
                    for m in range(2):
                        la = lnpool.tile([128, QT], BF16, tag="la")
                        lb = lnpool.tile([128, QT], BF16, tag="lb")
                        nc.sync.dma_start(la[:], a2a_out[ha][gp, 128 * m:128 * m + 128, :])
                        nc.sync.dma_start(lb[:], a2a_out[ha][gp + 4, 128 * m:128 * m + 128, :])
                        nc.vector.tensor_tensor(
                            out=gath[:, 2 * gp + m, :], in0=la[:], in1=lb[:], op=ADD
                        )
                for rc in range(2):
                    R = 2 * ha + rc  # local 128-row chunk index
                    y_sb = lnpool.tile([128, D], F32, tag="y")
                    res_sb = lnpool.tile([128, D], F32, tag="res")
                    nc.sync.dma_start(res_sb[:], resid[128 * R:128 * R + 128, :])
                    for n in range(2):
                        ps = psm.tile([128, 512], F32, tag="m")
                        for d2 in range(8):
                            nc.tensor.matmul(
                                ps[:],
                                lhsT=gath[:, d2, 128 * rc:128 * rc + 128],
                                rhs=wo_sb[:, d2, 512 * n:512 * n + 512],
                                start=(d2 == 0),
                                stop=(d2 == 7),
                            )
                        nc.vector.tensor_tensor(
                            out=y_sb[:, 512 * n:512 * n + 512],
                            in0=ps[:],
                            in1=res_sb[:, 512 * n:512 * n + 512],
                            op=ADD,
                        )
                    # LayerNorm over D: bn_stats mean/var + DVE rsqrt bit-trick
                    st = lnpool.tile([128, 16], F32, tag="st")
                    sti = lnpool.tile([128, 2], I32, tag="sti")
                    nc.vector.bn_stats(st[:, 0:6], y_sb[:, 0:512])
                    nc.vector.bn_stats(st[:, 6:12], y_sb[:, 512:1024])
                    nc.vector.bn_aggr(st[:, 12:14], st[:, 0:12])
                    mu = st[:, 12:13]
                    # v = var + eps; y0 = bitcast(0x5f3759df - (v_int >> 1))
                    nc.vector.tensor_tensor(
                        out=st[:, 14:15], in0=st[:, 13:14], in1=eps_sb, op=ADD
                    )
                    v = st[:, 14:15]
                    nc.vector.tensor_scalar(
                        out=sti[:, 0:1], in0=v.bitcast(I32), scalar1=1,
                        scalar2=None, op0=SHR,
                    )
                    nc.vector.tensor_scalar(
                        out=sti[:, 1:2], in0=sti[:, 0:1], scalar1=-1,
                        scalar2=0x5F3759DF, op0=MUL, op1=ADD,
                    )
                    y0 = sti[:, 1:2].bitcast(F32)
                    # h2 = -0.5 v ; two Newton steps: y <- y*(1.5 + h2*y*y)
                    nc.vector.tensor_scalar(
                        out=st[:, 15:16], in0=v, scalar1=-0.5, scalar2=None, op0=MUL
                    )
                    h2 = st[:, 15:16]
                    nc.vector.tensor_tensor(out=st[:, 0:1], in0=y0, in1=y0, op=MUL)
                    nc.vector.tensor_scalar(
                        out=st[:, 1:2], in0=st[:, 0:1], scalar1=h2, scalar2=half,
                        op0=MUL, op1=ADD,
                    )
                    nc.vector.tensor_tensor(out=st[:, 2:3], in0=y0, in1=st[:, 1:2], op=MUL)
                    nc.vector.tensor_tensor(
                        out=st[:, 3:4], in0=st[:, 2:3], in1=st[:, 2:3], op=MUL
                    )
                    nc.vector.tensor_scalar(
                        out=st[:, 4:5], in0=st[:, 3:4], scalar1=h2, scalar2=half,
                        op0=MUL, op1=ADD,
                    )
                    nc.vector.tensor_tensor(out=st[:, 5:6], in0=st[:, 2:3], in1=st[:, 4:5], op=MUL)
                    rstd = st[:, 5:6]
                    # yc = (y - mu) * rstd ; out = yc*gamma + beta
                    yc = lnpool.tile([128, D], F32, tag="yc")
                    nc.vector.tensor_scalar(
                        out=yc[:], in0=y_sb[:],
                        scalar1=mu, scalar2=rstd, op0=SUB, op1=MUL,
                    )
                    nc.vector.tensor_tensor(out=yc[:], in0=yc[:], in1=gam_sb[:], op=MUL)
                    nc.vector.tensor_tensor(out=yc[:], in0=yc[:], in1=bet_sb[:], op=ADD)
                    nc.sync.dma_start(out_d[128 * R:128 * R + 128, :], yc[:])

    nc.compile()
    return nc


def _prep_inputs(x_q, x_k, x_v, mask, Wq, bq, Wk, bk, Wv, bv, Wo, bo, gamma, beta):
    import ml_dtypes

    f = np.float32
    bf = ml_dtypes.bfloat16
    maskA = np.zeros((KB, QT), f)
    maskB = np.zeros((KB, QT), f)
    for i in range(KB):
        maskA[i, i:] = 1.0
        if i + 128 < QT:
            maskB[i, i + 128:] = 1.0
    mo = np.concatenate([maskA, maskB], axis=1).astype(bf)
    mo8 = np.concatenate([maskA, maskB], axis=1).astype(ml_dtypes.float8_e4m3)
    in_maps = []
    for c in range(NC):
        b, g = c // 4, c % 4
        dv = slice(DVC * g, DVC * (g + 1))
        smallc = np.zeros((128, 288), f)
        smallc[:, 0:2] = bq[dv].astype(f).reshape(2, 128).T
        smallc[:, 2:4] = bk[dv].astype(f).reshape(2, 128).T
        smallc[:, 4] = 1.0 - b
        smallc[:, 5] = float(b)
        smallc[:, 12] = EPS
        smallc[:, 13] = -2.0
        smallc[:, 16:16 + DVC] = np.broadcast_to(bv[dv].astype(f), (128, DVC))
        in_maps.append(
            {
                "xtq": np.ascontiguousarray(x_q[b].T.astype(bf)),
                "xtk": np.ascontiguousarray(x_k[b].T.astype(bf)),
                "xtv": np.ascontiguousarray(x_v[b].T.astype(bf)),
                "wqT": np.ascontiguousarray(Wq[dv, :].T.astype(bf)),
                "wkT": np.ascontiguousarray(Wk[dv, :].T.astype(bf)),
                "wvT": np.ascontiguousarray(Wv[dv, :].T.astype(bf)),
                "woT": np.ascontiguousarray(Wo.T.astype(bf)),
                "smallc": smallc,
                "gam_bc": np.broadcast_to(gamma.astype(f), (128, D)).copy(),
                "bet_bc": np.broadcast_to(beta.astype(f), (128, D)).copy(),
                "resid": np.ascontiguousarray(
                    x_q[b, 512 * g:512 * (g + 1), :].astype(f) + bo.astype(f)
                ),
                "mo": mo,
                "mo8": mo8,
                "ones_r": np.ones((1, 64), f),
            }
        )
    return in_maps


def kernel(x_q, x_k, x_v, mask, Wq, bq, Wk, bk, Wv, bv, Wo, bo, gamma, beta):
    _install_ntff_shim()
    from concourse.bass_utils import run_bass_kernel_spmd

    x_q, x_k, x_v = np.asarray(x_q), np.asarray(x_k), np.asarray(x_v)
    mask = np.asarray(mask)
    # this kernel implements causal attention structurally; verify the mask
    causal = np.tril(np.ones((S, S), mask.dtype))
    assert np.array_equal(mask.reshape(S, S), causal), "kernel specialized for causal mask"

    if "nc" not in _cache:
        _cache["nc"] = _build()
    nc = _cache["nc"]

    in_maps = _prep_inputs(
        x_q, x_k, x_v, mask,
        np.asarray(Wq), np.asarray(bq), np.asarray(Wk), np.asarray(bk),
        np.asarray(Wv), np.asarray(bv), np.asarray(Wo), np.asarray(bo),
        np.asarray(gamma), np.asarray(beta),
    )
    res = run_bass_kernel_spmd(nc, in_maps, list(range(NC)))
    _cache["last_results"] = res

    out = np.empty((B, S, D), np.float32)
    for c in range(NC):
        b, g = c // 4, c % 4
        out[b, 512 * g:512 * (g + 1), :] = res.results[c]["out"]
    return out


# revision 53
# speedup vs baseline: 1.1187x; 1.0250x over previous
# Trainium2 Bass kernel for nn_MultiHeadAttention_87024627352037.
#
# Full module: y = LayerNorm(x_q + (softmax(mask(QK^T/sqrt(nd))) V) Wo^T + bo)
# with Q/K/V projections of x_q/x_k/x_v. Shapes: B=2, S=2048, D=1024, H=16.
#
# Sharding (8 cores): core c = (batch b=c//4, head-quad g=c%4).
# Each core projects Q/K/V for its 4 heads (dv=256) over its batch and runs
# causal attention in a fully transposed layout (scoresT = K_T^T Q_T, no
# max-subtraction -- scores are O(1); softmax denominator via a ones-column
# in the PV matmul). Projections are streamed and interleaved with the
# attention q-tiles (processed 0,2,4,6,1,3,5,7) so the PE ramps early and
# stays busy. The ACT engine runs *only* Exp (no table reloads): the
# denominator reciprocal is computed on DVE and broadcast across partitions
# with a tiny f32r matmul; LayerNorm's rsqrt uses a DVE bit-trick + Newton
# steps. PSUM->SBUF fixups ride on the idle GPSIMD engine. A per-batch
# AllToAll (groups of 4) re-shards ctx from head-sharding to row-sharding;
# each core computes output projection + residual + LayerNorm for its 512
# rows. The host only slices, transposes, and concatenates numpy arrays.
import os
import sys
import types

import numpy as np

B, S, D, H = 2, 2048, 1024, 16
ND = D // H          # 64
NC = 8               # cores
HPC = H // 4         # 4 heads per core
DVC = HPC * ND       # 256 dv per core
QT = 256             # q tile
NQT = S // QT        # 8 q tiles
KB = 128             # k block
EPS = 1e-5
SCALE = 1.0 / np.sqrt(ND)

# iteration order: even tiles first so the even-parity AllToAll can fire at
# ~44% of the attention work and overlap the odd-tile compute.
ITERS = (0, 2, 4, 6, 1, 3, 5, 7)
# K/V 128-blocks projected at each iteration (front-loaded so tile t always
# has K/V blocks 0..2t+1 available).
KV_SCHED = {0: (0, 1), 2: (2, 3, 4, 5), 4: (6, 7, 8, 9), 6: (10, 11, 12, 13),
            1: (14, 15), 3: (), 5: (), 7: ()}

_cache = {}


def _install_ntff_shim():
    # antenv.axon_hooks is absent in this image; register the NTFF profile
    # hook so trace=True can capture HW exec time (harmless if unused).
    if "antenv.axon_hooks" in sys.modules:
        return
    mod = types.ModuleType("antenv.axon_hooks")
    mod._hook = None
    mod.set_axon_ntff_profile_hook = lambda h: setattr(mod, "_hook", h)
    mod.get_axon_ntff_profile_hook = lambda: mod._hook
    sys.modules["antenv.axon_hooks"] = mod
    try:
        import antenv

        antenv.axon_hooks = mod
        from trn_agent_boot.trn_boot import _ntff_profile_via_ctypes

        mod._hook = _ntff_profile_via_ctypes("/opt/axon/libaxon_pjrt.so")
    except Exception:
        pass


def _build():
    import concourse.bass as bass
    import concourse.mybir as mybir
    import concourse.tile as tile
    from concourse import bacc

    F32 = mybir.dt.float32
    F32R = mybir.dt.float32r
    BF16 = mybir.dt.bfloat16
    FP8 = mybir.dt.float8e4
    I32 = mybir.dt.int32
    ADD = mybir.AluOpType.add
    MUL = mybir.AluOpType.mult
    SUB = mybir.AluOpType.subtract
    SHR = mybir.AluOpType.logical_shift_right
    AF = mybir.ActivationFunctionType

    nc = bacc.Bacc("TRN2", target_bir_lowering=False, debug=False, num_devices=NC)

    def din(name, shape, dt=BF16):
        return nc.dram_tensor(name, shape, dt, kind="ExternalInput").ap()

    xtq = din("xtq", [D, S])
    xtk = din("xtk", [D, S])
    xtv = din("xtv", [D, S])
    wqT = din("wqT", [D, DVC])
    wkT = din("wkT", [D, DVC])
    wvT = din("wvT", [D, DVC])
    woT = din("woT", [D, D])
    smallc = din("smallc", [128, 288], F32)   # bq2|bk2|eps|pad|bv4x64(@16)
    gam_bc = din("gam_bc", [128, D], F32)
    bet_bc = din("bet_bc", [128, D], F32)
    resid = din("resid", [512, D], F32)       # x_q rows + bo (host pre-added)
    mo_in = din("mo", [128, 2 * QT])          # maskA|maskB (bf16)
    mo8_in = din("mo8", [128, 2 * QT], mybir.dt.float8e4)
    ones_r = din("ones_r", [1, 64], F32R)
    out_d = nc.dram_tensor("out", [512, D], F32, kind="ExternalOutput").ap()

    groups = [list(range(NC))]

    with nc.allow_low_precision(reason="f32r/bf16 matmul operand chain"), tile.TileContext(
        nc
    ) as tc:
        with (
            tc.tile_pool(name="const", bufs=1) as cpool,
            tc.tile_pool(name="res", bufs=1) as rpool,
            tc.tile_pool(name="xt", bufs=6) as xtpool,
            tc.tile_pool(name="pt", bufs=3) as ptpool,
            tc.tile_pool(name="dn", bufs=2) as dnpool,
            tc.tile_pool(name="gath", bufs=1) as gathpool,
            tc.tile_pool(name="ln", bufs=2) as lnpool,
            tc.tile_pool(name="ps_s", bufs=2, space="PSUM") as pss,
            tc.tile_pool(name="ps_ctx", bufs=2, space="PSUM") as psc,
            tc.tile_pool(name="ps_m", bufs=2, space="PSUM") as psm,
            tc.tile_pool(name="dram", bufs=1, space="DRAM") as dram,
        ):
            # ---- small constants + projection weights (needed first) ----
            smallc_sb = cpool.tile([128, 288], F32)
            mo_sb = cpool.tile([128, 2 * QT], BF16)
            mo8_sb = cpool.tile([128, 2 * QT], FP8)
            ones_sb = cpool.tile([1, 64], F32R)
            wq_sb = cpool.tile([128, 8, DVC], BF16)
            wk_sb = cpool.tile([128, 8, DVC], BF16)
            wv_sb = cpool.tile([128, 8, DVC], BF16)
            # spread startup loads across engine queues so issue overlaps
            nc.sync.dma_start(wk_sb[:], wkT.rearrange("(c p) n -> p c n", p=128))
            nc.scalar.dma_start(wq_sb[:], wqT.rearrange("(c p) n -> p c n", p=128))
            nc.gpsimd.dma_start(wv_sb[:], wvT.rearrange("(c p) n -> p c n", p=128))
            nc.sync.dma_start(smallc_sb[:], smallc)
            nc.sync.dma_start(ones_sb[:], ones_r)
            nc.sync.dma_start(mo_sb[:], mo_in)
            nc.sync.dma_start(mo8_sb[:], mo8_in)
            bq_sb = smallc_sb[:, 0:2]
            bk_sb = smallc_sb[:, 2:4]
            eps_sb = smallc_sb[:, 12:13]
            nbias_sb = smallc_sb[:, 13:14]  # -2.0 exp bias
            bv_sb = smallc_sb[:, 16:16 + DVC]   # bv broadcast (no ones col)
            mAB_sb = mo_sb[:, 0:2 * QT]

            # ---- resident activation tensors ----
            QT_sb = rpool.tile([128, 2, S], BF16)   # q^T: [dd(2x128), q]
            KT_sb = rpool.tile([128, 2, S], BF16)   # k^T: [dd(2x128), kpos]
            V_sb = rpool.tile([128, S // 128, HPC * (ND + 4)], FP8)
            ctx_sb = rpool.tile([128, 2, S], BF16)  # ctx^T: [dv(2x128), q]
            # ones columns of the V slots (denominator trick), set once;
            # slots are 68 wide (16B-aligned strides for dual-fp8 ldweights):
            # 64 data cols, a ones col, 3 zero pad cols.
            nc.gpsimd.memset(V_sb[:], 0.0)
            nc.gpsimd.memset(
                V_sb[:].rearrange("p c (h x) -> p c h x", x=ND + 4)[:, :, :, ND:ND + 1],
                1.0,
            )

            # ---- PE warm-up: the p-state ramp needs ~3us of continuous
            # execution to reach full clock. Burn the initial DMA wait on
            # dummy matmuls over the zero-initialized V_sb so the first real
            # projections start at 2.4GHz.
            warm = psm.tile([128, 512], F32, tag="m")
            for _w in range(24):
                nc.tensor.matmul(
                    warm[:],
                    lhsT=V_sb[:, 0, 0:128],
                    rhs=V_sb[:, 0:2, 0:256],
                    start=True,
                    stop=True,
                    skip_group_check=True,
                )

            # ---- heavyweight phase-3 constants: loaded later (see below) --
            wo_sb = cpool.tile([128, 8, D], BF16)
            gam_sb = cpool.tile([128, D], F32)
            bet_sb = cpool.tile([128, D], F32)

            # ---- A2A buffers ----
            # 8-slot AllToAll (groups of 4 unsupported): slot j carries the
            # rows of dest j%4 if j's batch matches ours, zeros otherwise;
            # receivers just add slot g' and g'+4.
            a2a_in = [
                dram.tile([NC, DVC, QT], BF16, name=f"a2a_in{i}") for i in range(2)
            ]
            a2a_out = [
                dram.tile([NC, DVC, QT], BF16, name=f"a2a_out{i}") for i in range(2)
            ]
            # zsel [128, 2]: col 0 = 1.0 if our batch is 0 else 0.0; col 1 =
            # the complement. The sender writes ctx*zsel0 into the batch-0
            # slot and ctx*zsel1 into the batch-1 slot -- one is the real
            # data, the other zeros -- so receivers just add the two slots
            # (no data-dependent addressing, no receive-side select).
            zsel_sb = smallc_sb[:, 4:6]

            def proj_kq(w_sb, xt_d, b_sb, o_sb, c0):
                # project 256 source columns [c0, c0+256) into o_sb (K^T/Q^T)
                xts = xtpool.tile([128, 8, 256], BF16, tag="xt")
                nc.sync.dma_start(
                    xts[:],
                    xt_d.rearrange("(c p) n -> p c n", p=128)[:, :, c0:c0 + 256],
                )
                for m in range(2):
                    ps = psm.tile([128, 512], F32, tag="m")
                    for cc in range(8):
                        nc.tensor.matmul(
                            ps[:, 0:256],
                            lhsT=w_sb[:, cc, 128 * m:128 * m + 128],
                            rhs=xts[:, cc, :],
                            start=(cc == 0),
                            stop=(cc == 7),
                        )
                    nc.vector.tensor_scalar(
                        out=o_sb[:, m, c0:c0 + 256],
                        in0=ps[:, 0:256],
                        scalar1=b_sb[:, m:m + 1],
                        scalar2=None,
                        op0=ADD,
                    )

            def proj_v(c0):
                # project V for k rows [c0, c0+256) (two 128-blocks)
                xvs = xtpool.tile([128, 8, 256], BF16, tag="xt")
                nc.sync.dma_start(
                    xvs[:],
                    xtv.rearrange("(c p) n -> p c n", p=128)[:, :, c0:c0 + 256],
                )
                for r in range(2):
                    rc = c0 // 128 + r
                    ps = psm.tile([128, 512], F32, tag="m")
                    for cc in range(8):
                        nc.tensor.matmul(
                            ps[:, 0:DVC],
                            lhsT=xvs[:, cc, 128 * r:128 * r + 128],
                            rhs=wv_sb[:, cc, :],
                            start=(cc == 0),
                            stop=(cc == 7),
                        )
                    v_slot = V_sb[:, rc, :].rearrange("p (h x) -> p h x", x=ND + 4)[
                        :, :, 0:ND
                    ]
                    nc.vector.tensor_tensor(
                        out=v_slot,
                        in0=ps[:, 0:DVC].rearrange("p (h x) -> p h x", x=ND),
                        in1=bv_sb.rearrange("p (h x) -> p h x", x=ND),
                        op=ADD,
                    )

            def emit_pv(h, grp, pt, ctxps, t):
                co = 256 * (h % 2)
                ptv = pt.rearrange("p (b q) -> p b q", q=256)
                for idx, jp in enumerate(grp):
                    nc.tensor.matmul(
                        ctxps[0:ND + 4, co:co + 256],
                        lhsT=V_sb[:, 2 * jp:2 * jp + 2, (ND + 4) * h:(ND + 4) * (h + 1)],
                        rhs=ptv[:, 2 * idx:2 * idx + 2, :],
                        start=(jp == 0),
                        stop=(jp == t),
                        perf_mode=mybir.MatmulPerfMode.DoubleRow,
                        skip_group_check=True,
                    )

            # Deferred per-tile epilogue: the denominator broadcast matmul,
            # the normalize-divides, and the ship DMAs of tile t run during
            # iteration t+1 so the PE never waits on the reciprocal chain.
            def finish_tile(pend):
                t, ctxps_pair, dn, rcp = pend
                for pi in range(2):
                    bps = psm.tile([128, 512], F32, tag="m")
                    nc.tensor.matmul(
                        bps[0:64, :],
                        lhsT=ones_sb[0:1, :],
                        rhs=dn[0:1, 512 * pi:512 * pi + 512],
                        start=True,
                        stop=True,
                    )
                    nc.vector.tensor_copy(
                        rcp[:, 512 * pi:512 * pi + 512],
                        bps[0:64, :],
                    )
                for h in range(HPC):
                    po = 64 * (h % 2)
                    hc = h // 2
                    co = 256 * (h % 2)
                    nc.vector.tensor_tensor(
                        out=ctx_sb[po:po + 64, hc, QT * t:QT * t + QT],
                        in0=ctxps_pair[h // 2][0:64, co:co + 256],
                        in1=rcp[:, 512 * (h // 2) + co:512 * (h // 2) + co + 256],
                        op=MUL,
                    )
                ha, cp = t % 2, t // 2
                ship = ptpool.tile([128, 2, 2, QT], BF16, tag="ship")
                for z in range(2):
                    nc.vector.tensor_scalar(
                        out=ship[:, z, :, :],
                        in0=ctx_sb[:, :, QT * t:QT * t + QT],
                        scalar1=zsel_sb[:, z:z + 1],
                        scalar2=None,
                        op0=MUL,
                    )
                for m in range(2):
                    nc.sync.dma_start(
                        a2a_in[ha][cp, 128 * m:128 * m + 128, :],
                        ship[:, 0, m, :],
                    )
                    nc.sync.dma_start(
                        a2a_in[ha][cp + 4, 128 * m:128 * m + 128, :],
                        ship[:, 1, m, :],
                    )
                if t == 6:
                    nc.gpsimd.collective_compute(
                        "AllToAll",
                        mybir.AluOpType.bypass,
                        replica_groups=groups,
                        ins=[a2a_in[0].opt()],
                        outs=[a2a_out[0].opt()],
                    )

            # ================= main loop =================
            pending = None
            for i, t in enumerate(ITERS):
                # ---- streamed projections for this iteration ----
                blocks = KV_SCHED[t]
                for p0 in range(0, len(blocks), 2):
                    c0 = blocks[p0] * 128
                    proj_kq(wk_sb, xtk, bk_sb, KT_sb, c0)
                    proj_v(c0)
                proj_kq(wq_sb, xtq, bq_sb, QT_sb, QT * t)
                if pending is not None:
                    finish_tile(pending)
                    pending = None
                if i == 4:
                    # phase-3 constants: load mid-flight, off the hot window
                    nc.sync.dma_start(
                        wo_sb[:], woT.rearrange("(c p) n -> p c n", p=128)
                    )
                    nc.sync.dma_start(gam_sb[:], gam_bc)
                    nc.sync.dma_start(bet_sb[:], bet_bc)

                # ---- attention for q-tile t ----
                ctxps_pair = []
                for _pi in range(2):
                    cpt = psc.tile([128, 512], F32, tag="c")
                    ctxps_pair.append(cpt)
                for h in range(HPC):
                    po = 64 * (h % 2)
                    hc = h // 2
                    ctxps = ctxps_pair[h // 2]
                    q_rhs = QT_sb[po:po + 64, hc, QT * t:QT * t + QT]
                    jps = list(range(t + 1))
                    grps = [tuple(jps[k:k + 2]) for k in range(0, len(jps), 2)]
                    prev = None
                    for grp in grps:
                        w = 512 * len(grp)
                        sps = pss.tile([128, 1024], F32, tag="s")
                        for idx, jp in enumerate(grp):
                            for u in range(2):
                                nc.tensor.matmul(
                                    sps[:, 256 * (2 * idx + u):256 * (2 * idx + u) + 256],
                                    lhsT=KT_sb[
                                        po:po + 64,
                                        hc,
                                        128 * (2 * jp + u):128 * (2 * jp + u) + 128,
                                    ],
                                    rhs=q_rhs,
                                    start=True,
                                    stop=True,
                                )
                        pt = ptpool.tile([128, 1024], FP8, tag="pt")
                        # bias -2 keeps exp() under fp8e4 max; it cancels
                        # in softmax (the ones-column denominator sums the
                        # same fp8 values).
                        nc.scalar.activation(
                            pt[:, 0:w], sps[:, 0:w], AF.Exp, scale=SCALE, bias=nbias_sb
                        )
                        if t in grp:
                            do = 512 * grp.index(t)
                            nc.vector.tensor_tensor(
                                out=pt[:, do:do + 512],
                                in0=pt[:, do:do + 512],
                                in1=mo8_sb,
                                op=MUL,
                            )
                        if prev is not None:
                            emit_pv(h, prev[0], prev[1], ctxps, t)
                        prev = (grp, pt)
                    emit_pv(h, prev[0], prev[1], ctxps, t)

                # ---- denominator reciprocal chain (ACT/DVE, overlaps PV) ----
                dcp = dnpool.tile([1, 1024], F32, tag="dcp")
                dn0 = dnpool.tile([1, 1024], F32, tag="dn0")
                dn = dnpool.tile([1, 1024], F32R, tag="dn")
                rcp = dnpool.tile([64, 1024], F32, tag="rcp")
                for pi in range(2):
                    nc.scalar.activation(
                        dcp[0:1, 512 * pi:512 * pi + 512],
                        ctxps_pair[pi][64:65, 0:512],
                        AF.Copy,
                    )
                    nc.vector.reciprocal_approx_fast(
                        out=dn0[0:1, 512 * pi:512 * pi + 512],
                        in_=dcp[0:1, 512 * pi:512 * pi + 512],
                    )
                    nc.vector.tensor_copy(
                        dn[0:1, 512 * pi:512 * pi + 512],
                        dn0[0:1, 512 * pi:512 * pi + 512],
                    )
                pending = (t, ctxps_pair, dn, rcp)

            # tile 7's epilogue + collective #1 dispatch FIRST, so the
            # collective's flight overlaps the ha=0 output projection.
            finish_tile(pending)
            nc.gpsimd.collective_compute(
                "AllToAll",
                mybir.AluOpType.bypass,
                replica_groups=groups,
                ins=[a2a_in[1].opt()],
                outs=[a2a_out[1].opt()],
            )

            # ---- phase 3: gather + output projection + residual + LN ----
            half = 1.5
            for ha in range(2):
                gath = gathpool.tile([128, 8, QT], BF16, tag=f"gath{ha}")
                for gp in range(4):
                    for m in range(2):
                        la = lnpool.tile([128, QT], BF16, tag="la")
                        lb = lnpool.tile([128, QT], BF16, tag="lb")
                        nc.sync.dma_start(la[:], a2a_out[ha][gp, 128 * m:128 * m + 128, :])
                        nc.sync.dma_start(lb[:], a2a_out[ha][gp + 4, 128 * m:128 * m + 128, :])
                        nc.vector.tensor_tensor(
                            out=gath[:, 2 * gp + m, :], in0=la[:], in1=lb[:], op=ADD
                        )
                for rc in range(2):
                    R = 2 * ha + rc  # local 128-row chunk index
                    y_sb = lnpool.tile([128, D], F32, tag="y")
                    res_sb = lnpool.tile([128, D], F32, tag="res")
                    nc.sync.dma_start(res_sb[:], resid[128 * R:128 * R + 128, :])
                    for n in range(2):
                        ps = psm.tile([128, 512], F32, tag="m")
                        for d2 in range(8):
                            nc.tensor.matmul(
                                ps[:],
                                lhsT=gath[:, d2, 128 * rc:128 * rc + 128],
                                rhs=wo_sb[:, d2, 512 * n:512 * n + 512],
                                start=(d2 == 0),
                                stop=(d2 == 7),
                            )
                        nc.vector.tensor_tensor(
                            out=y_sb[:, 512 * n:512 * n + 512],
                            in0=ps[:],
                            in1=res_sb[:, 512 * n:512 * n + 512],
                            op=ADD,
                        )
                    # LayerNorm over D: bn_stats mean/var + DVE rsqrt bit-trick
                    st = lnpool.tile([128, 16], F32, tag="st")
                    sti = lnpool.tile([128, 2], I32, tag="sti")
                    nc.vector.bn_stats(st[:, 0:6], y_sb[:, 0:512])
                    nc.vector.bn_stats(st[:, 6:12], y_sb[:, 512:1024])
                    nc.vector.bn_aggr(st[:, 12:14], st[:, 0:12])
                    mu = st[:, 12:13]
                    # v = var + eps; y0 = bitcast(0x5f3759df - (v_int >> 1))
                    nc.vector.tensor_tensor(
                        out=st[:, 14:15], in0=st[:, 13:14], in1=eps_sb, op=ADD
                    )
                    v = st[:, 14:15]
                    nc.vector.tensor_scalar(
                        out=sti[:, 0:1], in0=v.bitcast(I32), scalar1=1,
                        scalar2=None, op0=SHR,
                    )
                    nc.vector.tensor_scalar(
                        out=sti[:, 1:2], in0=sti[:, 0:1], scalar1=-1,
                        scalar2=0x5F3759DF, op0=MUL, op1=ADD,
                    )
                    y0 = sti[:, 1:2].bitcast(F32)
                    # h2 = -0.5 v ; two Newton steps: y <- y*(1.5 + h2*y*y)
                    nc.vector.tensor_scalar(
                        out=st[:, 15:16], in0=v, scalar1=-0.5, scalar2=None, op0=MUL
                    )
                    h2 = st[:, 15:16]
                    nc.vector.tensor_tensor(out=st[:, 0:1], in0=y0, in1=y0, op=MUL)
                    nc.vector.tensor_scalar(
                        out=st[:, 1:2], in0=st[:, 0:1], scalar1=h2, scalar2=half,
                        op0=MUL, op1=ADD,
                    )
                    nc.vector.tensor_tensor(out=st[:, 2:3], in0=y0, in1=st[:, 1:2], op=MUL)
                    nc.vector.tensor_tensor(
                        out=st[:, 3:4], in0=st[:, 2:3], in1=st[:, 2:3], op=MUL
                    )
                    nc.vector.tensor_scalar(
                        out=st[:, 4:5], in0=st[:, 3:4], scalar1=h2, scalar2=half,
                        op0=MUL, op1=ADD,
                    )
                    nc.vector.tensor_tensor(out=st[:, 5:6], in0=st[:, 2:3], in1=st[:, 4:5], op=MUL)
                    rstd = st[:, 5:6]
                    # yc = (y - mu) * rstd ; out = yc*gamma + beta
                    yc = lnpool.tile([128, D], F32, tag="yc")
                    nc.vector.tensor_scalar(
                        out=yc[:], in0=y_sb[:],
                        scalar1=mu, scalar2=rstd, op0=SUB, op1=MUL,
                    )
                    nc.vector.tensor_tensor(out=yc[:], in0=yc[:], in1=gam_sb[:], op=MUL)
                    nc.vector.tensor_tensor(out=yc[:], in0=yc[:], in1=bet_sb[:], op=ADD)
                    nc.sync.dma_start(out_d[128 * R:128 * R + 128, :], yc[:])

    nc.compile()
    return nc


def _prep_inputs(x_q, x_k, x_v, mask, Wq, bq, Wk, bk, Wv, bv, Wo, bo, gamma, beta):
    import ml_dtypes

    f = np.float32
    bf = ml_dtypes.bfloat16
    maskA = np.zeros((KB, QT), f)
    maskB = np.zeros((KB, QT), f)
    for i in range(KB):
        maskA[i, i:] = 1.0
        if i + 128 < QT:
            maskB[i, i + 128:] = 1.0
    mo = np.concatenate([maskA, maskB], axis=1).astype(bf)
    mo8 = np.concatenate([maskA, maskB], axis=1).astype(ml_dtypes.float8_e4m3)
    in_maps = []
    for c in range(NC):
        b, g = c // 4, c % 4
        dv = slice(DVC * g, DVC * (g + 1))
        smallc = np.zeros((128, 288), f)
        smallc[:, 0:2] = bq[dv].astype(f).reshape(2, 128).T
        smallc[:, 2:4] = bk[dv].astype(f).reshape(2, 128).T
        smallc[:, 4] = 1.0 - b
        smallc[:, 5] = float(b)
        smallc[:, 12] = EPS
        smallc[:, 13] = -2.0
        smallc[:, 16:16 + DVC] = np.broadcast_to(bv[dv].astype(f), (128, DVC))
        in_maps.append(
            {
                "xtq": np.ascontiguousarray(x_q[b].T.astype(bf)),
                "xtk": np.ascontiguousarray(x_k[b].T.astype(bf)),
                "xtv": np.ascontiguousarray(x_v[b].T.astype(bf)),
                "wqT": np.ascontiguousarray(Wq[dv, :].T.astype(bf)),
                "wkT": np.ascontiguousarray(Wk[dv, :].T.astype(bf)),
                "wvT": np.ascontiguousarray(Wv[dv, :].T.astype(bf)),
                "woT": np.ascontiguousarray(Wo.T.astype(bf)),
                "smallc": smallc,
                "gam_bc": np.broadcast_to(gamma.astype(f), (128, D)).copy(),
                "bet_bc": np.broadcast_to(beta.astype(f), (128, D)).copy(),
                "resid": np.ascontiguousarray(
                    x_q[b, 512 * g:512 * (g + 1), :].astype(f) + bo.astype(f)
                ),
                "mo": mo,
                "mo8": mo8,
                "ones_r": np.ones((1, 64), f),
            }
        )
    return in_maps


def kernel(x_q, x_k, x_v, mask, Wq, bq, Wk, bk, Wv, bv, Wo, bo, gamma, beta):
    _install_ntff_shim()
    from concourse.bass_utils import run_bass_kernel_spmd

    x_q, x_k, x_v = np.asarray(x_q), np.asarray(x_k), np.asarray(x_v)
    mask = np.asarray(mask)
    # this kernel implements causal attention structurally; verify the mask
    causal = np.tril(np.ones((S, S), mask.dtype))
    assert np.array_equal(mask.reshape(S, S), causal), "kernel specialized for causal mask"

    if "nc" not in _cache:
        _cache["nc"] = _build()
    nc = _cache["nc"]

    in_maps = _prep_inputs(
        x_q, x_k, x_v, mask,
        np.asarray(Wq), np.asarray(bq), np.asarray(Wk), np.asarray(bk),
        np.asarray(Wv), np.asarray(bv), np.asarray(Wo), np.asarray(bo),
        np.asarray(gamma), np.asarray(beta),
    )
    res = run_bass_kernel_spmd(nc, in_maps, list(range(NC)))
    _cache["last_results"] = res

    out = np.empty((B, S, D), np.float32)
    for c in range(NC):
        b, g = c // 4, c % 4
        out[b, 512 * g:512 * (g + 1), :] = res.results[c]["out"]
    return out


# revision 54
# speedup vs baseline: 1.1754x; 1.0507x over previous
# Trainium2 Bass kernel for nn_MultiHeadAttention_87024627352037.
#
# Full module: y = LayerNorm(x_q + (softmax(mask(QK^T/sqrt(nd))) V) Wo^T + bo)
# with Q/K/V projections of x_q/x_k/x_v. Shapes: B=2, S=2048, D=1024, H=16.
#
# Sharding (8 cores): core c = (batch b=c//4, head-quad g=c%4).
# Each core projects Q/K/V for its 4 heads (dv=256) over its batch and runs
# causal attention in a fully transposed layout (scoresT = K_T^T Q_T, no
# max-subtraction -- scores are O(1); softmax denominator via a ones-column
# in the PV matmul). Projections are streamed and interleaved with the
# attention q-tiles (processed 0,2,4,6,1,3,5,7) so the PE ramps early and
# stays busy. The ACT engine runs *only* Exp (no table reloads): the
# denominator reciprocal is computed on DVE and broadcast across partitions
# with a tiny f32r matmul; LayerNorm's rsqrt uses a DVE bit-trick + Newton
# steps. PSUM->SBUF fixups ride on the idle GPSIMD engine. A per-batch
# AllToAll (groups of 4) re-shards ctx from head-sharding to row-sharding;
# each core computes output projection + residual + LayerNorm for its 512
# rows. The host only slices, transposes, and concatenates numpy arrays.
import os
import sys
import types

import numpy as np

B, S, D, H = 2, 2048, 1024, 16
ND = D // H          # 64
NC = 8               # cores
HPC = H // 4         # 4 heads per core
DVC = HPC * ND       # 256 dv per core
QT = 256             # q tile
NQT = S // QT        # 8 q tiles
KB = 128             # k block
EPS = 1e-5
SCALE = 1.0 / np.sqrt(ND)

# iteration order: even tiles first so the even-parity AllToAll can fire at
# ~44% of the attention work and overlap the odd-tile compute.
ITERS = (0, 2, 4, 6, 1, 3, 5, 7)
# K/V 128-blocks projected at each iteration (front-loaded so tile t always
# has K/V blocks 0..2t+1 available).
KV_SCHED = {0: (0, 1), 2: (2, 3, 4, 5), 4: (6, 7, 8, 9), 6: (10, 11, 12, 13),
            1: (14, 15), 3: (), 5: (), 7: ()}

_cache = {}


def _install_ntff_shim():
    # antenv.axon_hooks is absent in this image; register the NTFF profile
    # hook so trace=True can capture HW exec time (harmless if unused).
    if "antenv.axon_hooks" in sys.modules:
        return
    mod = types.ModuleType("antenv.axon_hooks")
    mod._hook = None
    mod.set_axon_ntff_profile_hook = lambda h: setattr(mod, "_hook", h)
    mod.get_axon_ntff_profile_hook = lambda: mod._hook
    sys.modules["antenv.axon_hooks"] = mod
    try:
        import antenv

        antenv.axon_hooks = mod
        from trn_agent_boot.trn_boot import _ntff_profile_via_ctypes

        mod._hook = _ntff_profile_via_ctypes("/opt/axon/libaxon_pjrt.so")
    except Exception:
        pass


def _build():
    import concourse.bass as bass
    import concourse.mybir as mybir
    import concourse.tile as tile
    from concourse import bacc

    F32 = mybir.dt.float32
    F32R = mybir.dt.float32r
    BF16 = mybir.dt.bfloat16
    FP8 = mybir.dt.float8e4
    I32 = mybir.dt.int32
    ADD = mybir.AluOpType.add
    MUL = mybir.AluOpType.mult
    SUB = mybir.AluOpType.subtract
    SHR = mybir.AluOpType.logical_shift_right
    AF = mybir.ActivationFunctionType

    nc = bacc.Bacc("TRN2", target_bir_lowering=False, debug=False, num_devices=NC)

    def din(name, shape, dt=BF16):
        return nc.dram_tensor(name, shape, dt, kind="ExternalInput").ap()

    xtq = din("xtq", [D, S])
    xtk = din("xtk", [D, S])
    xtv = din("xtv", [D, S])
    wqT = din("wqT", [D, DVC])
    wkT = din("wkT", [D, DVC])
    wvT = din("wvT", [D, DVC])
    woT = din("woT", [D, D])
    smallc = din("smallc", [128, 288], F32)   # bq2|bk2|eps|pad|bv4x64(@16)
    gam_bc = din("gam_bc", [128, D], F32)
    bet_bc = din("bet_bc", [128, D], F32)
    resid = din("resid", [512, D], F32)       # x_q rows + bo (host pre-added)
    mo_in = din("mo", [128, 2 * QT])          # maskA|maskB (bf16)
    mo8_in = din("mo8", [128, 2 * QT], mybir.dt.float8e4)
    ones_r = din("ones_r", [1, 64], F32R)
    out_d = nc.dram_tensor("out", [512, D], F32, kind="ExternalOutput").ap()

    groups = [list(range(NC))]

    with nc.allow_low_precision(reason="f32r/bf16 matmul operand chain"), tile.TileContext(
        nc
    ) as tc:
        with (
            tc.tile_pool(name="const", bufs=1) as cpool,
            tc.tile_pool(name="res", bufs=1) as rpool,
            tc.tile_pool(name="xt", bufs=6) as xtpool,
            tc.tile_pool(name="pt", bufs=3) as ptpool,
            tc.tile_pool(name="dn", bufs=2) as dnpool,
            tc.tile_pool(name="gath", bufs=1) as gathpool,
            tc.tile_pool(name="ln", bufs=2) as lnpool,
            tc.tile_pool(name="ps_s", bufs=2, space="PSUM") as pss,
            tc.tile_pool(name="ps_ctx", bufs=2, space="PSUM") as psc,
            tc.tile_pool(name="ps_m", bufs=2, space="PSUM") as psm,
            tc.tile_pool(name="dram", bufs=1, space="DRAM") as dram,
        ):
            # ---- small constants + projection weights (needed first) ----
            smallc_sb = cpool.tile([128, 288], F32)
            mo_sb = cpool.tile([128, 2 * QT], BF16)
            mo8_sb = cpool.tile([128, 2 * QT], FP8)
            ones_sb = cpool.tile([1, 64], F32R)
            wq_sb = cpool.tile([128, 8, DVC], BF16)
            wk_sb = cpool.tile([128, 8, DVC], BF16)
            wv_sb = cpool.tile([128, 8, DVC], BF16)
            # spread startup loads across engine queues so issue overlaps
            nc.sync.dma_start(wk_sb[:], wkT.rearrange("(c p) n -> p c n", p=128))
            nc.scalar.dma_start(wq_sb[:], wqT.rearrange("(c p) n -> p c n", p=128))
            nc.gpsimd.dma_start(wv_sb[:], wvT.rearrange("(c p) n -> p c n", p=128))
            nc.sync.dma_start(smallc_sb[:], smallc)
            nc.sync.dma_start(ones_sb[:], ones_r)
            nc.sync.dma_start(mo_sb[:], mo_in)
            nc.sync.dma_start(mo8_sb[:], mo8_in)
            bq_sb = smallc_sb[:, 0:2]
            bk_sb = smallc_sb[:, 2:4]
            eps_sb = smallc_sb[:, 12:13]
            nbias_sb = smallc_sb[:, 13:14]  # -2.0 exp bias
            bv_sb = smallc_sb[:, 16:16 + DVC]   # bv broadcast (no ones col)
            mAB_sb = mo_sb[:, 0:2 * QT]

            # ---- resident activation tensors ----
            QT_sb = rpool.tile([128, 2, S], BF16)   # q^T: [dd(2x128), q]
            KT_sb = rpool.tile([128, 2, S], BF16)   # k^T: [dd(2x128), kpos]
            V_sb = rpool.tile([128, S // 128, HPC * (ND + 4)], FP8)
            ctx_sb = rpool.tile([128, 2, S], BF16)  # ctx^T: [dv(2x128), q]
            # ones columns of the V slots (denominator trick), set once;
            # slots are 68 wide (16B-aligned strides for dual-fp8 ldweights):
            # 64 data cols, a ones col, 3 zero pad cols.
            nc.gpsimd.memset(V_sb[:], 0.0)
            nc.gpsimd.memset(
                V_sb[:].rearrange("p c (h x) -> p c h x", x=ND + 4)[:, :, :, ND:ND + 1],
                1.0,
            )

            # ---- heavyweight phase-3 constants: loaded later (see below) --
            wo_sb = cpool.tile([128, 8, D], BF16)
            gam_sb = cpool.tile([128, D], F32)
            bet_sb = cpool.tile([128, D], F32)

            # ---- A2A buffers ----
            # 8-slot AllToAll (groups of 4 unsupported): slot j carries the
            # rows of dest j%4 if j's batch matches ours, zeros otherwise;
            # receivers just add slot g' and g'+4.
            a2a_in = [
                dram.tile([NC, DVC, QT], BF16, name=f"a2a_in{i}") for i in range(2)
            ]
            a2a_out = [
                dram.tile([NC, DVC, QT], BF16, name=f"a2a_out{i}") for i in range(2)
            ]
            # zsel [128, 2]: col 0 = 1.0 if our batch is 0 else 0.0; col 1 =
            # the complement. The sender writes ctx*zsel0 into the batch-0
            # slot and ctx*zsel1 into the batch-1 slot -- one is the real
            # data, the other zeros -- so receivers just add the two slots
            # (no data-dependent addressing, no receive-side select).
            zsel_sb = smallc_sb[:, 4:6]

            def proj_kq(w_sb, xt_d, b_sb, o_sb, c0):
                # project 256 source columns [c0, c0+256) into o_sb (K^T/Q^T)
                xts = xtpool.tile([128, 8, 256], BF16, tag="xt")
                nc.sync.dma_start(
                    xts[:],
                    xt_d.rearrange("(c p) n -> p c n", p=128)[:, :, c0:c0 + 256],
                )
                for m in range(2):
                    ps = psm.tile([128, 512], F32, tag="m")
                    for cc in range(8):
                        nc.tensor.matmul(
                            ps[:, 0:256],
                            lhsT=w_sb[:, cc, 128 * m:128 * m + 128],
                            rhs=xts[:, cc, :],
                            start=(cc == 0),
                            stop=(cc == 7),
                        )
                    nc.vector.tensor_scalar(
                        out=o_sb[:, m, c0:c0 + 256],
                        in0=ps[:, 0:256],
                        scalar1=b_sb[:, m:m + 1],
                        scalar2=None,
                        op0=ADD,
                    )

            def proj_v(c0):
                # project V for k rows [c0, c0+256) (two 128-blocks)
                xvs = xtpool.tile([128, 8, 256], BF16, tag="xt")
                nc.sync.dma_start(
                    xvs[:],
                    xtv.rearrange("(c p) n -> p c n", p=128)[:, :, c0:c0 + 256],
                )
                for r in range(2):
                    rc = c0 // 128 + r
                    ps = psm.tile([128, 512], F32, tag="m")
                    for cc in range(8):
                        nc.tensor.matmul(
                            ps[:, 0:DVC],
                            lhsT=xvs[:, cc, 128 * r:128 * r + 128],
                            rhs=wv_sb[:, cc, :],
                            start=(cc == 0),
                            stop=(cc == 7),
                        )
                    v_slot = V_sb[:, rc, :].rearrange("p (h x) -> p h x", x=ND + 4)[
                        :, :, 0:ND
                    ]
                    nc.vector.tensor_tensor(
                        out=v_slot,
                        in0=ps[:, 0:DVC].rearrange("p (h x) -> p h x", x=ND),
                        in1=bv_sb.rearrange("p (h x) -> p h x", x=ND),
                        op=ADD,
                    )

            def emit_pv(h, grp, pt, ctxps, t):
                co = 256 * (h % 2)
                ptv = pt.rearrange("p (b q) -> p b q", q=256)
                for idx, jp in enumerate(grp):
                    nc.tensor.matmul(
                        ctxps[0:ND + 4, co:co + 256],
                        lhsT=V_sb[:, 2 * jp:2 * jp + 2, (ND + 4) * h:(ND + 4) * (h + 1)],
                        rhs=ptv[:, 2 * idx:2 * idx + 2, :],
                        start=(jp == 0),
                        stop=(jp == t),
                        perf_mode=mybir.MatmulPerfMode.DoubleRow,
                        skip_group_check=True,
                    )

            # Deferred per-tile epilogue: the denominator broadcast matmul,
            # the normalize-divides, and the ship DMAs of tile t run during
            # iteration t+1 so the PE never waits on the reciprocal chain.
            def finish_tile(pend):
                t, ctxps_pair, dn, rcp = pend
                for pi in range(2):
                    bps = psm.tile([128, 512], F32, tag="m")
                    nc.tensor.matmul(
                        bps[0:64, :],
                        lhsT=ones_sb[0:1, :],
                        rhs=dn[0:1, 512 * pi:512 * pi + 512],
                        start=True,
                        stop=True,
                    )
                    nc.vector.tensor_copy(
                        rcp[:, 512 * pi:512 * pi + 512],
                        bps[0:64, :],
                    )
                for h in range(HPC):
                    po = 64 * (h % 2)
                    hc = h // 2
                    co = 256 * (h % 2)
                    nc.vector.tensor_tensor(
                        out=ctx_sb[po:po + 64, hc, QT * t:QT * t + QT],
                        in0=ctxps_pair[h // 2][0:64, co:co + 256],
                        in1=rcp[:, 512 * (h // 2) + co:512 * (h // 2) + co + 256],
                        op=MUL,
                    )
                ha, cp = t % 2, t // 2
                ship = ptpool.tile([128, 2, 2, QT], BF16, tag="ship")
                for z in range(2):
                    nc.vector.tensor_scalar(
                        out=ship[:, z, :, :],
                        in0=ctx_sb[:, :, QT * t:QT * t + QT],
                        scalar1=zsel_sb[:, z:z + 1],
                        scalar2=None,
                        op0=MUL,
                    )
                for m in range(2):
                    nc.sync.dma_start(
                        a2a_in[ha][cp, 128 * m:128 * m + 128, :],
                        ship[:, 0, m, :],
                    )
                    nc.sync.dma_start(
                        a2a_in[ha][cp + 4, 128 * m:128 * m + 128, :],
                        ship[:, 1, m, :],
                    )
                if t == 6:
                    nc.gpsimd.collective_compute(
                        "AllToAll",
                        mybir.AluOpType.bypass,
                        replica_groups=groups,
                        ins=[a2a_in[0].opt()],
                        outs=[a2a_out[0].opt()],
                    )

            # ================= main loop =================
            pending = None
            for i, t in enumerate(ITERS):
                # ---- streamed projections for this iteration ----
                blocks = KV_SCHED[t]
                for p0 in range(0, len(blocks), 2):
                    c0 = blocks[p0] * 128
                    proj_kq(wk_sb, xtk, bk_sb, KT_sb, c0)
                    proj_v(c0)
                proj_kq(wq_sb, xtq, bq_sb, QT_sb, QT * t)
                if pending is not None:
                    finish_tile(pending)
                    pending = None
                if i == 4:
                    # phase-3 constants: load mid-flight, off the hot window
                    nc.sync.dma_start(
                        wo_sb[:], woT.rearrange("(c p) n -> p c n", p=128)
                    )
                    nc.sync.dma_start(gam_sb[:], gam_bc)
                    nc.sync.dma_start(bet_sb[:], bet_bc)

                # ---- attention for q-tile t ----
                ctxps_pair = []
                for _pi in range(2):
                    cpt = psc.tile([128, 512], F32, tag="c")
                    ctxps_pair.append(cpt)
                for h in range(HPC):
                    po = 64 * (h % 2)
                    hc = h // 2
                    ctxps = ctxps_pair[h // 2]
                    q_rhs = QT_sb[po:po + 64, hc, QT * t:QT * t + QT]
                    jps = list(range(t + 1))
                    grps = [tuple(jps[k:k + 2]) for k in range(0, len(jps), 2)]
                    prev = None
                    for grp in grps:
                        w = 512 * len(grp)
                        sps = pss.tile([128, 1024], F32, tag="s")
                        for idx, jp in enumerate(grp):
                            for u in range(2):
                                nc.tensor.matmul(
                                    sps[:, 256 * (2 * idx + u):256 * (2 * idx + u) + 256],
                                    lhsT=KT_sb[
                                        po:po + 64,
                                        hc,
                                        128 * (2 * jp + u):128 * (2 * jp + u) + 128,
                                    ],
                                    rhs=q_rhs,
                                    start=True,
                                    stop=True,
                                )
                        pt = ptpool.tile([128, 1024], FP8, tag="pt")
                        # bias -2 keeps exp() under fp8e4 max; it cancels
                        # in softmax (the ones-column denominator sums the
                        # same fp8 values).
                        nc.scalar.activation(
                            pt[:, 0:w], sps[:, 0:w], AF.Exp, scale=SCALE, bias=nbias_sb
                        )
                        if t in grp:
                            do = 512 * grp.index(t)
                            nc.vector.tensor_tensor(
                                out=pt[:, do:do + 512],
                                in0=pt[:, do:do + 512],
                                in1=mo8_sb,
                                op=MUL,
                            )
                        if prev is not None:
                            emit_pv(h, prev[0], prev[1], ctxps, t)
                        prev = (grp, pt)
                    emit_pv(h, prev[0], prev[1], ctxps, t)

                # ---- denominator reciprocal chain (ACT/DVE, overlaps PV) ----
                dcp = dnpool.tile([1, 1024], F32, tag="dcp")
                dn0 = dnpool.tile([1, 1024], F32, tag="dn0")
                dn = dnpool.tile([1, 1024], F32R, tag="dn")
                rcp = dnpool.tile([64, 1024], F32, tag="rcp")
                for pi in range(2):
                    nc.scalar.activation(
                        dcp[0:1, 512 * pi:512 * pi + 512],
                        ctxps_pair[pi][64:65, 0:512],
                        AF.Copy,
                    )
                    nc.vector.reciprocal_approx_fast(
                        out=dn0[0:1, 512 * pi:512 * pi + 512],
                        in_=dcp[0:1, 512 * pi:512 * pi + 512],
                    )
                    nc.vector.tensor_copy(
                        dn[0:1, 512 * pi:512 * pi + 512],
                        dn0[0:1, 512 * pi:512 * pi + 512],
                    )
                pending = (t, ctxps_pair, dn, rcp)

            # tile 7's epilogue + collective #1 dispatch FIRST, so the
            # collective's flight overlaps the ha=0 output projection.
            finish_tile(pending)
            nc.gpsimd.collective_compute(
                "AllToAll",
                mybir.AluOpType.bypass,
                replica_groups=groups,
                ins=[a2a_in[1].opt()],
                outs=[a2a_out[1].opt()],
            )

            # ---- phase 3: gather + output projection + residual + LN ----
            half = 1.5
            for ha in range(2):
                gath = gathpool.tile([128, 8, QT], BF16, tag=f"gath{ha}")
                for gp in range(4):
                    for m in range(2):
                        la = lnpool.tile([128, QT], BF16, tag="la")
                        lb = lnpool.tile([128, QT], BF16, tag="lb")
                        nc.sync.dma_start(la[:], a2a_out[ha][gp, 128 * m:128 * m + 128, :])
                        nc.sync.dma_start(lb[:], a2a_out[ha][gp + 4, 128 * m:128 * m + 128, :])
                        nc.vector.tensor_tensor(
                            out=gath[:, 2 * gp + m, :], in0=la[:], in1=lb[:], op=ADD
                        )
                for rc in range(2):
                    R = 2 * ha + rc  # local 128-row chunk index
                    y_sb = lnpool.tile([128, D], F32, tag="y")
                    res_sb = lnpool.tile([128, D], F32, tag="res")
                    nc.sync.dma_start(res_sb[:], resid[128 * R:128 * R + 128, :])
                    for n in range(2):
                        ps = psm.tile([128, 512], F32, tag="m")
                        for d2 in range(8):
                            nc.tensor.matmul(
                                ps[:],
                                lhsT=gath[:, d2, 128 * rc:128 * rc + 128],
                                rhs=wo_sb[:, d2, 512 * n:512 * n + 512],
                                start=(d2 == 0),
                                stop=(d2 == 7),
                            )
                        nc.vector.tensor_tensor(
                            out=y_sb[:, 512 * n:512 * n + 512],
                            in0=ps[:],
                            in1=res_sb[:, 512 * n:512 * n + 512],
                            op=ADD,
                        )
                    # LayerNorm over D: bn_stats mean/var + DVE rsqrt bit-trick
                    st = lnpool.tile([128, 16], F32, tag="st")
                    sti = lnpool.tile([128, 2], I32, tag="sti")
                    nc.vector.bn_stats(st[:, 0:6], y_sb[:, 0:512])
                    nc.vector.bn_stats(st[:, 6:12], y_sb[:, 512:1024])
                    nc.vector.bn_aggr(st[:, 12:14], st[:, 0:12])
                    mu = st[:, 12:13]
                    # v = var + eps; y0 = bitcast(0x5f3759df - (v_int >> 1))
                    nc.vector.tensor_tensor(
                        out=st[:, 14:15], in0=st[:, 13:14], in1=eps_sb, op=ADD
                    )
                    v = st[:, 14:15]
                    nc.vector.tensor_scalar(
                        out=sti[:, 0:1], in0=v.bitcast(I32), scalar1=1,
                        scalar2=None, op0=SHR,
                    )
                    nc.vector.tensor_scalar(
                        out=sti[:, 1:2], in0=sti[:, 0:1], scalar1=-1,
                        scalar2=0x5F3759DF, op0=MUL, op1=ADD,
                    )
                    y0 = sti[:, 1:2].bitcast(F32)
                    # h2 = -0.5 v ; two Newton steps: y <- y*(1.5 + h2*y*y)
                    nc.vector.tensor_scalar(
                        out=st[:, 15:16], in0=v, scalar1=-0.5, scalar2=None, op0=MUL
                    )
                    h2 = st[:, 15:16]
                    nc.vector.tensor_tensor(out=st[:, 0:1], in0=y0, in1=y0, op=MUL)
                    nc.vector.tensor_scalar(
                        out=st[:, 1:2], in0=st[:, 0:1], scalar1=h2, scalar2=half,
                        op0=MUL, op1=ADD,
                    )
                    nc.vector.tensor_tensor(out=st[:, 2:3], in0=y0, in1=st[:, 1:2], op=MUL)
                    nc.vector.tensor_tensor(
                        out=st[:, 3:4], in0=st[:, 2:3], in1=st[:, 2:3], op=MUL
                    )
                    nc.vector.tensor_scalar(
                        out=st[:, 4:5], in0=st[:, 3:4], scalar1=h2, scalar2=half,
                        op0=MUL, op1=ADD,
                    )
                    nc.vector.tensor_tensor(out=st[:, 5:6], in0=st[:, 2:3], in1=st[:, 4:5], op=MUL)
                    rstd = st[:, 5:6]
                    # yc = (y - mu) * rstd ; out = yc*gamma + beta
                    yc = lnpool.tile([128, D], F32, tag="yc")
                    nc.vector.tensor_scalar(
                        out=yc[:], in0=y_sb[:],
                        scalar1=mu, scalar2=rstd, op0=SUB, op1=MUL,
                    )
                    nc.vector.tensor_tensor(out=yc[:], in0=yc[:], in1=gam_sb[:], op=MUL)
                    nc.vector.tensor_tensor(out=yc[:], in0=yc[:], in1=bet_sb[:], op=ADD)
                    nc.sync.dma_start(out_d[128 * R:128 * R + 128, :], yc[:])

    nc.compile()
    return nc


def _prep_inputs(x_q, x_k, x_v, mask, Wq, bq, Wk, bk, Wv, bv, Wo, bo, gamma, beta):
    import ml_dtypes

    f = np.float32
    bf = ml_dtypes.bfloat16
    maskA = np.zeros((KB, QT), f)
    maskB = np.zeros((KB, QT), f)
    for i in range(KB):
        maskA[i, i:] = 1.0
        if i + 128 < QT:
            maskB[i, i + 128:] = 1.0
    mo = np.concatenate([maskA, maskB], axis=1).astype(bf)
    mo8 = np.concatenate([maskA, maskB], axis=1).astype(ml_dtypes.float8_e4m3)
    in_maps = []
    for c in range(NC):
        b, g = c // 4, c % 4
        dv = slice(DVC * g, DVC * (g + 1))
        smallc = np.zeros((128, 288), f)
        smallc[:, 0:2] = bq[dv].astype(f).reshape(2, 128).T
        smallc[:, 2:4] = bk[dv].astype(f).reshape(2, 128).T
        smallc[:, 4] = 1.0 - b
        smallc[:, 5] = float(b)
        smallc[:, 12] = EPS
        smallc[:, 13] = -2.0
        smallc[:, 16:16 + DVC] = np.broadcast_to(bv[dv].astype(f), (128, DVC))
        in_maps.append(
            {
                "xtq": np.ascontiguousarray(x_q[b].T.astype(bf)),
                "xtk": np.ascontiguousarray(x_k[b].T.astype(bf)),
                "xtv": np.ascontiguousarray(x_v[b].T.astype(bf)),
                "wqT": np.ascontiguousarray(Wq[dv, :].T.astype(bf)),
                "wkT": np.ascontiguousarray(Wk[dv, :].T.astype(bf)),
                "wvT": np.ascontiguousarray(Wv[dv, :].T.astype(bf)),
                "woT": np.ascontiguousarray(Wo.T.astype(bf)),
                "smallc": smallc,
                "gam_bc": np.broadcast_to(gamma.astype(f), (128, D)).copy(),
                "bet_bc": np.broadcast_to(beta.astype(f), (128, D)).copy(),
                "resid": np.ascontiguousarray(
                    x_q[b, 512 * g:512 * (g + 1), :].astype(f) + bo.astype(f)
                ),
                "mo": mo,
                "mo8": mo8,
                "ones_r": np.ones((1, 64), f),
            }
        )
    return in_maps


def kernel(x_q, x_k, x_v, mask, Wq, bq, Wk, bk, Wv, bv, Wo, bo, gamma, beta):
    _install_ntff_shim()
    from concourse.bass_utils import run_bass_kernel_spmd

    x_q, x_k, x_v = np.asarray(x_q), np.asarray(x_k), np.asarray(x_v)
    mask = np.asarray(mask)
    # this kernel implements causal attention structurally; verify the mask
    causal = np.tril(np.ones((S, S), mask.dtype))
    assert np.array_equal(mask.reshape(S, S), causal), "kernel specialized for causal mask"

    if "nc" not in _cache:
        _cache["nc"] = _build()
    nc = _cache["nc"]

    in_maps = _prep_inputs(
        x_q, x_k, x_v, mask,
        np.asarray(Wq), np.asarray(bq), np.asarray(Wk), np.asarray(bk),
        np.asarray(Wv), np.asarray(bv), np.asarray(Wo), np.asarray(bo),
        np.asarray(gamma), np.asarray(beta),
    )
    res = run_bass_kernel_spmd(nc, in_maps, list(range(NC)))
    _cache["last_results"] = res

    out = np.empty((B, S, D), np.float32)
    for c in range(NC):
        b, g = c // 4, c % 4
        out[b, 512 * g:512 * (g + 1), :] = res.results[c]["out"]
    return out


# revision 56
# speedup vs baseline: 1.2436x; 1.0580x over previous
# Trainium2 Bass kernel for nn_MultiHeadAttention_87024627352037.
#
# Full module: y = LayerNorm(x_q + (softmax(mask(QK^T/sqrt(nd))) V) Wo^T + bo)
# with Q/K/V projections of x_q/x_k/x_v. Shapes: B=2, S=2048, D=1024, H=16.
#
# Sharding (8 cores): core c = (batch b=c//4, head-quad g=c%4).
# Each core projects Q/K/V for its 4 heads (dv=256) over its batch and runs
# causal attention in a fully transposed layout (scoresT = K_T^T Q_T, no
# max-subtraction -- scores are O(1); softmax denominator via a ones-column
# in the PV matmul). Projections are streamed and interleaved with the
# attention q-tiles (processed 0,2,4,6,1,3,5,7) so the PE ramps early and
# stays busy. The ACT engine runs *only* Exp (no table reloads): the
# denominator reciprocal is computed on DVE and broadcast across partitions
# with a tiny f32r matmul; LayerNorm's rsqrt uses a DVE bit-trick + Newton
# steps. PSUM->SBUF fixups ride on the idle GPSIMD engine. A per-batch
# AllToAll (groups of 4) re-shards ctx from head-sharding to row-sharding;
# each core computes output projection + residual + LayerNorm for its 512
# rows. The host only slices, transposes, and concatenates numpy arrays.
import os
import sys
import types

import numpy as np

B, S, D, H = 2, 2048, 1024, 16
ND = D // H          # 64
NC = 8               # cores
HPC = H // 4         # 4 heads per core
DVC = HPC * ND       # 256 dv per core
QT = 256             # q tile
NQT = S // QT        # 8 q tiles
KB = 128             # k block
EPS = 1e-5
SCALE = 1.0 / np.sqrt(ND)

# iteration order: even tiles first so the even-parity AllToAll can fire at
# ~44% of the attention work and overlap the odd-tile compute.
ITERS = (0, 2, 4, 6, 1, 3, 5, 7)
# K/V 128-blocks projected at each iteration (front-loaded so tile t always
# has K/V blocks 0..2t+1 available).
KV_SCHED = {0: (0, 1), 2: (2, 3, 4, 5), 4: (6, 7, 8, 9), 6: (10, 11, 12, 13),
            1: (14, 15), 3: (), 5: (), 7: ()}

_cache = {}


def _install_ntff_shim():
    # antenv.axon_hooks is absent in this image; register the NTFF profile
    # hook so trace=True can capture HW exec time (harmless if unused).
    if "antenv.axon_hooks" in sys.modules:
        return
    mod = types.ModuleType("antenv.axon_hooks")
    mod._hook = None
    mod.set_axon_ntff_profile_hook = lambda h: setattr(mod, "_hook", h)
    mod.get_axon_ntff_profile_hook = lambda: mod._hook
    sys.modules["antenv.axon_hooks"] = mod
    try:
        import antenv

        antenv.axon_hooks = mod
        from trn_agent_boot.trn_boot import _ntff_profile_via_ctypes

        mod._hook = _ntff_profile_via_ctypes("/opt/axon/libaxon_pjrt.so")
    except Exception:
        pass


def _build():
    import concourse.bass as bass
    import concourse.mybir as mybir
    import concourse.tile as tile
    from concourse import bacc

    F32 = mybir.dt.float32
    F32R = mybir.dt.float32r
    BF16 = mybir.dt.bfloat16
    FP8 = mybir.dt.float8e4
    I32 = mybir.dt.int32
    ADD = mybir.AluOpType.add
    MUL = mybir.AluOpType.mult
    SUB = mybir.AluOpType.subtract
    SHR = mybir.AluOpType.logical_shift_right
    AF = mybir.ActivationFunctionType

    nc = bacc.Bacc("TRN2", target_bir_lowering=False, debug=False, num_devices=NC)

    def din(name, shape, dt=BF16):
        return nc.dram_tensor(name, shape, dt, kind="ExternalInput").ap()

    xtq = din("xtq", [D, S])
    xtk = din("xtk", [D, S])
    xtv = din("xtv", [D, S])
    wqT = din("wqT", [D, DVC])
    wkT = din("wkT", [D, DVC])
    wvT = din("wvT", [D, DVC])
    woT = din("woT", [D, D])
    smallc = din("smallc", [128, 288], F32)   # bq2|bk2|eps|pad|bv4x64(@16)
    gam_bc = din("gam_bc", [128, D], F32)
    bet_bc = din("bet_bc", [128, D], F32)
    resid = din("resid", [512, D], F32)       # x_q rows + bo (host pre-added)
    mo_in = din("mo", [128, 2 * QT])          # maskA|maskB (bf16)
    mo8_in = din("mo8", [128, 2 * QT], mybir.dt.float8e4)
    ones_r = din("ones_r", [1, 64], F32R)
    out_d = nc.dram_tensor("out", [512, D], F32, kind="ExternalOutput").ap()

    groups = [list(range(NC))]

    with nc.allow_low_precision(reason="f32r/bf16 matmul operand chain"), tile.TileContext(
        nc
    ) as tc:
        with (
            tc.tile_pool(name="const", bufs=1) as cpool,
            tc.tile_pool(name="res", bufs=1) as rpool,
            tc.tile_pool(name="xt", bufs=6) as xtpool,
            tc.tile_pool(name="pt", bufs=3) as ptpool,
            tc.tile_pool(name="dn", bufs=2) as dnpool,
            tc.tile_pool(name="gath", bufs=1) as gathpool,
            tc.tile_pool(name="ln", bufs=2) as lnpool,
            tc.tile_pool(name="ps_s", bufs=2, space="PSUM") as pss,
            tc.tile_pool(name="ps_ctx", bufs=2, space="PSUM") as psc,
            tc.tile_pool(name="ps_m", bufs=2, space="PSUM") as psm,
            tc.tile_pool(name="dram", bufs=1, space="DRAM") as dram,
        ):
            # ---- small constants + projection weights (needed first) ----
            smallc_sb = cpool.tile([128, 288], F32)
            mo_sb = cpool.tile([128, 2 * QT], BF16)
            mo8_sb = cpool.tile([128, 2 * QT], FP8)
            ones_sb = cpool.tile([1, 64], F32R)
            wq_sb = cpool.tile([128, 8, DVC], BF16)
            wk_sb = cpool.tile([128, 8, DVC], BF16)
            wv_sb = cpool.tile([128, 8, DVC], BF16)
            # spread startup loads across engine queues so issue overlaps
            nc.sync.dma_start(wk_sb[:], wkT.rearrange("(c p) n -> p c n", p=128))
            nc.scalar.dma_start(wq_sb[:], wqT.rearrange("(c p) n -> p c n", p=128))
            nc.gpsimd.dma_start(wv_sb[:], wvT.rearrange("(c p) n -> p c n", p=128))
            nc.sync.dma_start(smallc_sb[:], smallc)
            nc.sync.dma_start(ones_sb[:], ones_r)
            nc.sync.dma_start(mo_sb[:], mo_in)
            nc.sync.dma_start(mo8_sb[:], mo8_in)
            bq_sb = smallc_sb[:, 0:2]
            bk_sb = smallc_sb[:, 2:4]
            eps_sb = smallc_sb[:, 12:13]
            nbias_sb = smallc_sb[:, 13:14]  # -2.0 exp bias
            bv_sb = smallc_sb[:, 16:16 + DVC]   # bv broadcast (no ones col)
            mAB_sb = mo_sb[:, 0:2 * QT]

            # ---- resident activation tensors ----
            QT_sb = rpool.tile([128, 2, S], BF16)   # q^T: [dd(2x128), q]
            KT_sb = rpool.tile([128, 2, S], BF16)   # k^T: [dd(2x128), kpos]
            V_sb = rpool.tile([128, S // 128, HPC * (ND + 4)], FP8)
            ctx_sb = rpool.tile([128, 2, S], BF16)  # ctx^T: [dv(2x128), q]
            # ones columns of the V slots (denominator trick), set once;
            # slots are 68 wide (16B-aligned strides for dual-fp8 ldweights):
            # 64 data cols, a ones col, 3 zero pad cols.
            nc.gpsimd.memset(V_sb[:], 0.0)
            nc.gpsimd.memset(
                V_sb[:].rearrange("p c (h x) -> p c h x", x=ND + 4)[:, :, :, ND:ND + 1],
                1.0,
            )

            # ---- heavyweight phase-3 constants: loaded later (see below) --
            wo_sb = cpool.tile([128, 8, D], BF16)
            gam_sb = cpool.tile([128, D], F32)
            bet_sb = cpool.tile([128, D], F32)

            # ---- A2A buffers ----
            # Row ownership is interleaved at 128-row granularity across BOTH
            # batches: core j owns rows [512*(j//2)+128*(j%2), +128) of each
            # batch (even-tile set, parity 0) plus the same +256 (odd set).
            # Every A2A slot then carries real data -- no batch-dup zeros, no
            # receive-side select -- at half the previous payload.
            a2a_in = [
                dram.tile([NC, DVC, 128], BF16, name=f"a2a_in{i}") for i in range(2)
            ]
            a2a_out = [
                dram.tile([NC, DVC, 128], BF16, name=f"a2a_out{i}") for i in range(2)
            ]

            def proj_kq(w_sb, xt_d, b_sb, o_sb, c0):
                # project 256 source columns [c0, c0+256) into o_sb (K^T/Q^T)
                xts = xtpool.tile([128, 8, 256], BF16, tag="xt")
                nc.sync.dma_start(
                    xts[:],
                    xt_d.rearrange("(c p) n -> p c n", p=128)[:, :, c0:c0 + 256],
                )
                for m in range(2):
                    ps = psm.tile([128, 512], F32, tag="m")
                    for cc in range(8):
                        nc.tensor.matmul(
                            ps[:, 0:256],
                            lhsT=w_sb[:, cc, 128 * m:128 * m + 128],
                            rhs=xts[:, cc, :],
                            start=(cc == 0),
                            stop=(cc == 7),
                        )
                    nc.vector.tensor_scalar(
                        out=o_sb[:, m, c0:c0 + 256],
                        in0=ps[:, 0:256],
                        scalar1=b_sb[:, m:m + 1],
                        scalar2=None,
                        op0=ADD,
                    )

            def proj_v(c0):
                # project V for k rows [c0, c0+256) (two 128-blocks)
                xvs = xtpool.tile([128, 8, 256], BF16, tag="xt")
                nc.sync.dma_start(
                    xvs[:],
                    xtv.rearrange("(c p) n -> p c n", p=128)[:, :, c0:c0 + 256],
                )
                for r in range(2):
                    rc = c0 // 128 + r
                    ps = psm.tile([128, 512], F32, tag="m")
                    for cc in range(8):
                        nc.tensor.matmul(
                            ps[:, 0:DVC],
                            lhsT=xvs[:, cc, 128 * r:128 * r + 128],
                            rhs=wv_sb[:, cc, :],
                            start=(cc == 0),
                            stop=(cc == 7),
                        )
                    v_slot = V_sb[:, rc, :].rearrange("p (h x) -> p h x", x=ND + 4)[
                        :, :, 0:ND
                    ]
                    nc.vector.tensor_tensor(
                        out=v_slot,
                        in0=ps[:, 0:DVC].rearrange("p (h x) -> p h x", x=ND),
                        in1=bv_sb.rearrange("p (h x) -> p h x", x=ND),
                        op=ADD,
                    )

            def emit_pv(h, grp, pt, ctxps, t):
                co = 256 * (h % 2)
                ptv = pt.rearrange("p (b q) -> p b q", q=256)
                for idx, jp in enumerate(grp):
                    nc.tensor.matmul(
                        ctxps[0:ND + 4, co:co + 256],
                        lhsT=V_sb[:, 2 * jp:2 * jp + 2, (ND + 4) * h:(ND + 4) * (h + 1)],
                        rhs=ptv[:, 2 * idx:2 * idx + 2, :],
                        start=(jp == 0),
                        stop=(jp == t),
                        perf_mode=mybir.MatmulPerfMode.DoubleRow,
                        skip_group_check=True,
                    )

            # Deferred per-tile epilogue: the denominator broadcast matmul,
            # the normalize-divides, and the ship DMAs of tile t run during
            # iteration t+1 so the PE never waits on the reciprocal chain.
            def finish_tile(pend):
                t, ctxps_pair, dn, rcp = pend
                for pi in range(2):
                    bps = psm.tile([128, 512], F32, tag="m")
                    nc.tensor.matmul(
                        bps[0:64, :],
                        lhsT=ones_sb[0:1, :],
                        rhs=dn[0:1, 512 * pi:512 * pi + 512],
                        start=True,
                        stop=True,
                    )
                    nc.vector.tensor_copy(
                        rcp[:, 512 * pi:512 * pi + 512],
                        bps[0:64, :],
                    )
                for h in range(HPC):
                    po = 64 * (h % 2)
                    hc = h // 2
                    co = 256 * (h % 2)
                    nc.vector.tensor_tensor(
                        out=ctx_sb[po:po + 64, hc, QT * t:QT * t + QT],
                        in0=ctxps_pair[h // 2][0:64, co:co + 256],
                        in1=rcp[:, 512 * (h // 2) + co:512 * (h // 2) + co + 256],
                        op=MUL,
                    )
                ha = t % 2
                for hh in range(2):
                    dest = (t - ha) + hh
                    for m in range(2):
                        nc.sync.dma_start(
                            a2a_in[ha][dest, 128 * m:128 * m + 128, :],
                            ctx_sb[:, m, QT * t + 128 * hh:QT * t + 128 * hh + 128],
                        )
                if t == 6:
                    nc.gpsimd.collective_compute(
                        "AllToAll",
                        mybir.AluOpType.bypass,
                        replica_groups=groups,
                        ins=[a2a_in[0].opt()],
                        outs=[a2a_out[0].opt()],
                    )

            # ================= main loop =================
            pending = None
            for i, t in enumerate(ITERS):
                # ---- streamed projections for this iteration ----
                blocks = KV_SCHED[t]
                for p0 in range(0, len(blocks), 2):
                    c0 = blocks[p0] * 128
                    proj_kq(wk_sb, xtk, bk_sb, KT_sb, c0)
                    proj_v(c0)
                proj_kq(wq_sb, xtq, bq_sb, QT_sb, QT * t)
                if pending is not None:
                    finish_tile(pending)
                    pending = None
                if i == 4:
                    # phase-3 constants: load mid-flight, off the hot window
                    nc.sync.dma_start(
                        wo_sb[:], woT.rearrange("(c p) n -> p c n", p=128)
                    )
                    nc.sync.dma_start(gam_sb[:], gam_bc)
                    nc.sync.dma_start(bet_sb[:], bet_bc)

                # ---- attention for q-tile t ----
                ctxps_pair = []
                for _pi in range(2):
                    cpt = psc.tile([128, 512], F32, tag="c")
                    ctxps_pair.append(cpt)
                for h in range(HPC):
                    po = 64 * (h % 2)
                    hc = h // 2
                    ctxps = ctxps_pair[h // 2]
                    q_rhs = QT_sb[po:po + 64, hc, QT * t:QT * t + QT]
                    jps = list(range(t + 1))
                    grps = [tuple(jps[k:k + 2]) for k in range(0, len(jps), 2)]
                    prev = None
                    for grp in grps:
                        w = 512 * len(grp)
                        sps = pss.tile([128, 1024], F32, tag="s")
                        for idx, jp in enumerate(grp):
                            for u in range(2):
                                nc.tensor.matmul(
                                    sps[:, 256 * (2 * idx + u):256 * (2 * idx + u) + 256],
                                    lhsT=KT_sb[
                                        po:po + 64,
                                        hc,
                                        128 * (2 * jp + u):128 * (2 * jp + u) + 128,
                                    ],
                                    rhs=q_rhs,
                                    start=True,
                                    stop=True,
                                )
                        pt = ptpool.tile([128, 1024], FP8, tag="pt")
                        # bias -2 keeps exp() under fp8e4 max; it cancels
                        # in softmax (the ones-column denominator sums the
                        # same fp8 values).
                        nc.scalar.activation(
                            pt[:, 0:w], sps[:, 0:w], AF.Exp, scale=SCALE, bias=nbias_sb
                        )
                        if t in grp:
                            do = 512 * grp.index(t)
                            nc.vector.tensor_tensor(
                                out=pt[:, do:do + 512],
                                in0=pt[:, do:do + 512],
                                in1=mo8_sb,
                                op=MUL,
                            )
                        if prev is not None:
                            emit_pv(h, prev[0], prev[1], ctxps, t)
                        prev = (grp, pt)
                    emit_pv(h, prev[0], prev[1], ctxps, t)

                # ---- denominator reciprocal chain (ACT/DVE, overlaps PV) ----
                dcp = dnpool.tile([1, 1024], F32, tag="dcp")
                dn0 = dnpool.tile([1, 1024], F32, tag="dn0")
                dn = dnpool.tile([1, 1024], F32R, tag="dn")
                rcp = dnpool.tile([64, 1024], F32, tag="rcp")
                for pi in range(2):
                    nc.scalar.activation(
                        dcp[0:1, 512 * pi:512 * pi + 512],
                        ctxps_pair[pi][64:65, 0:512],
                        AF.Copy,
                    )
                    nc.vector.reciprocal_approx_fast(
                        out=dn0[0:1, 512 * pi:512 * pi + 512],
                        in_=dcp[0:1, 512 * pi:512 * pi + 512],
                    )
                    nc.vector.tensor_copy(
                        dn[0:1, 512 * pi:512 * pi + 512],
                        dn0[0:1, 512 * pi:512 * pi + 512],
                    )
                pending = (t, ctxps_pair, dn, rcp)

            # tile 7's epilogue + collective #1 dispatch FIRST, so the
            # collective's flight overlaps the ha=0 output projection.
            finish_tile(pending)
            nc.gpsimd.collective_compute(
                "AllToAll",
                mybir.AluOpType.bypass,
                replica_groups=groups,
                ins=[a2a_in[1].opt()],
                outs=[a2a_out[1].opt()],
            )

            # ---- phase 3: gather + output projection + residual + LN ----
            half = 1.5
            for ha in range(2):
                gath = gathpool.tile([128, 2, 8, 128], BF16, tag=f"gath{ha}")
                for bb in range(2):
                    for gp in range(4):
                        for m in range(2):
                            nc.sync.dma_start(
                                gath[:, bb, 2 * gp + m, :],
                                a2a_out[ha][4 * bb + gp, 128 * m:128 * m + 128, :],
                            )
                for rc in range(2):
                    R = 2 * ha + rc  # local 128-row chunk index (batch rc)
                    y_sb = lnpool.tile([128, D], F32, tag="y")
                    res_sb = lnpool.tile([128, D], F32, tag="res")
                    nc.sync.dma_start(res_sb[:], resid[128 * R:128 * R + 128, :])
                    for n in range(2):
                        ps = psm.tile([128, 512], F32, tag="m")
                        for d2 in range(8):
                            nc.tensor.matmul(
                                ps[:],
                                lhsT=gath[:, rc, d2, :],
                                rhs=wo_sb[:, d2, 512 * n:512 * n + 512],
                                start=(d2 == 0),
                                stop=(d2 == 7),
                            )
                        nc.vector.tensor_tensor(
                            out=y_sb[:, 512 * n:512 * n + 512],
                            in0=ps[:],
                            in1=res_sb[:, 512 * n:512 * n + 512],
                            op=ADD,
                        )
                    # LayerNorm over D: bn_stats mean/var + DVE rsqrt bit-trick
                    st = lnpool.tile([128, 16], F32, tag="st")
                    sti = lnpool.tile([128, 2], I32, tag="sti")
                    nc.vector.bn_stats(st[:, 0:6], y_sb[:, 0:512])
                    nc.vector.bn_stats(st[:, 6:12], y_sb[:, 512:1024])
                    nc.vector.bn_aggr(st[:, 12:14], st[:, 0:12])
                    mu = st[:, 12:13]
                    # v = var + eps; y0 = bitcast(0x5f3759df - (v_int >> 1))
                    nc.vector.tensor_tensor(
                        out=st[:, 14:15], in0=st[:, 13:14], in1=eps_sb, op=ADD
                    )
                    v = st[:, 14:15]
                    nc.vector.tensor_scalar(
                        out=sti[:, 0:1], in0=v.bitcast(I32), scalar1=1,
                        scalar2=None, op0=SHR,
                    )
                    nc.vector.tensor_scalar(
                        out=sti[:, 1:2], in0=sti[:, 0:1], scalar1=-1,
                        scalar2=0x5F3759DF, op0=MUL, op1=ADD,
                    )
                    y0 = sti[:, 1:2].bitcast(F32)
                    # h2 = -0.5 v ; two Newton steps: y <- y*(1.5 + h2*y*y)
                    nc.vector.tensor_scalar(
                        out=st[:, 15:16], in0=v, scalar1=-0.5, scalar2=None, op0=MUL
                    )
                    h2 = st[:, 15:16]
                    nc.vector.tensor_tensor(out=st[:, 0:1], in0=y0, in1=y0, op=MUL)
                    nc.vector.tensor_scalar(
                        out=st[:, 1:2], in0=st[:, 0:1], scalar1=h2, scalar2=half,
                        op0=MUL, op1=ADD,
                    )
                    nc.vector.tensor_tensor(out=st[:, 2:3], in0=y0, in1=st[:, 1:2], op=MUL)
                    nc.vector.tensor_tensor(
                        out=st[:, 3:4], in0=st[:, 2:3], in1=st[:, 2:3], op=MUL
                    )
                    nc.vector.tensor_scalar(
                        out=st[:, 4:5], in0=st[:, 3:4], scalar1=h2, scalar2=half,
                        op0=MUL, op1=ADD,
                    )
                    nc.vector.tensor_tensor(out=st[:, 5:6], in0=st[:, 2:3], in1=st[:, 4:5], op=MUL)
                    rstd = st[:, 5:6]
                    # yc = (y - mu) * rstd ; out = yc*gamma + beta
                    yc = lnpool.tile([128, D], F32, tag="yc")
                    nc.vector.tensor_scalar(
                        out=yc[:], in0=y_sb[:],
                        scalar1=mu, scalar2=rstd, op0=SUB, op1=MUL,
                    )
                    nc.vector.tensor_tensor(out=yc[:], in0=yc[:], in1=gam_sb[:], op=MUL)
                    nc.vector.tensor_tensor(out=yc[:], in0=yc[:], in1=bet_sb[:], op=ADD)
                    nc.sync.dma_start(out_d[128 * R:128 * R + 128, :], yc[:])

    nc.compile()
    return nc


def _prep_inputs(x_q, x_k, x_v, mask, Wq, bq, Wk, bk, Wv, bv, Wo, bo, gamma, beta):
    import ml_dtypes

    f = np.float32
    bf = ml_dtypes.bfloat16
    maskA = np.zeros((KB, QT), f)
    maskB = np.zeros((KB, QT), f)
    for i in range(KB):
        maskA[i, i:] = 1.0
        if i + 128 < QT:
            maskB[i, i + 128:] = 1.0
    mo = np.concatenate([maskA, maskB], axis=1).astype(bf)
    mo8 = np.concatenate([maskA, maskB], axis=1).astype(ml_dtypes.float8_e4m3)
    in_maps = []
    for c in range(NC):
        b, g = c // 4, c % 4
        dv = slice(DVC * g, DVC * (g + 1))
        # interleaved cross-batch row ownership (see A2A comment in _build)
        re = 512 * (c // 2) + 128 * (c % 2)
        ro = re + 256
        smallc = np.zeros((128, 288), f)
        smallc[:, 0:2] = bq[dv].astype(f).reshape(2, 128).T
        smallc[:, 2:4] = bk[dv].astype(f).reshape(2, 128).T
        smallc[:, 4] = 1.0 - b
        smallc[:, 5] = float(b)
        smallc[:, 12] = EPS
        smallc[:, 13] = -2.0
        smallc[:, 16:16 + DVC] = np.broadcast_to(bv[dv].astype(f), (128, DVC))
        in_maps.append(
            {
                "xtq": np.ascontiguousarray(x_q[b].T.astype(bf)),
                "xtk": np.ascontiguousarray(x_k[b].T.astype(bf)),
                "xtv": np.ascontiguousarray(x_v[b].T.astype(bf)),
                "wqT": np.ascontiguousarray(Wq[dv, :].T.astype(bf)),
                "wkT": np.ascontiguousarray(Wk[dv, :].T.astype(bf)),
                "wvT": np.ascontiguousarray(Wv[dv, :].T.astype(bf)),
                "woT": np.ascontiguousarray(Wo.T.astype(bf)),
                "smallc": smallc,
                "gam_bc": np.broadcast_to(gamma.astype(f), (128, D)).copy(),
                "bet_bc": np.broadcast_to(beta.astype(f), (128, D)).copy(),
                "resid": np.ascontiguousarray(
                    np.concatenate(
                        [
                            x_q[0, re:re + 128, :],
                            x_q[1, re:re + 128, :],
                            x_q[0, ro:ro + 128, :],
                            x_q[1, ro:ro + 128, :],
                        ]
                    ).astype(f)
                    + bo.astype(f)
                ),
                "mo": mo,
                "mo8": mo8,
                "ones_r": np.ones((1, 64), f),
            }
        )
    return in_maps


def kernel(x_q, x_k, x_v, mask, Wq, bq, Wk, bk, Wv, bv, Wo, bo, gamma, beta):
    _install_ntff_shim()
    from concourse.bass_utils import run_bass_kernel_spmd

    x_q, x_k, x_v = np.asarray(x_q), np.asarray(x_k), np.asarray(x_v)
    mask = np.asarray(mask)
    # this kernel implements causal attention structurally; verify the mask
    causal = np.tril(np.ones((S, S), mask.dtype))
    assert np.array_equal(mask.reshape(S, S), causal), "kernel specialized for causal mask"

    if "nc" not in _cache:
        _cache["nc"] = _build()
    nc = _cache["nc"]

    in_maps = _prep_inputs(
        x_q, x_k, x_v, mask,
        np.asarray(Wq), np.asarray(bq), np.asarray(Wk), np.asarray(bk),
        np.asarray(Wv), np.asarray(bv), np.asarray(Wo), np.asarray(bo),
        np.asarray(gamma), np.asarray(beta),
    )
    res = run_bass_kernel_spmd(nc, in_maps, list(range(NC)))
    _cache["last_results"] = res

    out = np.empty((B, S, D), np.float32)
    for c in range(NC):
        re = 512 * (c // 2) + 128 * (c % 2)
        ro = re + 256
        r = res.results[c]["out"]
        out[0, re:re + 128, :] = r[0:128]
        out[1, re:re + 128, :] = r[128:256]
        out[0, ro:ro + 128, :] = r[256:384]
        out[1, ro:ro + 128, :] = r[384:512]
    return out


# revision 57
# speedup vs baseline: 1.2458x; 1.0018x over previous
# Trainium2 Bass kernel for nn_MultiHeadAttention_87024627352037.
#
# Full module: y = LayerNorm(x_q + (softmax(mask(QK^T/sqrt(nd))) V) Wo^T + bo)
# with Q/K/V projections of x_q/x_k/x_v. Shapes: B=2, S=2048, D=1024, H=16.
#
# Sharding (8 cores): core c = (batch b=c//4, head-quad g=c%4).
# Each core projects Q/K/V for its 4 heads (dv=256) over its batch and runs
# causal attention in a fully transposed layout (scoresT = K_T^T Q_T, no
# max-subtraction -- scores are O(1); softmax denominator via a ones-column
# in the PV matmul). Projections are streamed and interleaved with the
# attention q-tiles (processed 0,2,4,6,1,3,5,7) so the PE ramps early and
# stays busy. The ACT engine runs *only* Exp (no table reloads): the
# denominator reciprocal is computed on DVE and broadcast across partitions
# with a tiny f32r matmul; LayerNorm's rsqrt uses a DVE bit-trick + Newton
# steps. PSUM->SBUF fixups ride on the idle GPSIMD engine. A per-batch
# AllToAll (groups of 4) re-shards ctx from head-sharding to row-sharding;
# each core computes output projection + residual + LayerNorm for its 512
# rows. The host only slices, transposes, and concatenates numpy arrays.
import os
import sys
import types

import numpy as np

B, S, D, H = 2, 2048, 1024, 16
ND = D // H          # 64
NC = 8               # cores
HPC = H // 4         # 4 heads per core
DVC = HPC * ND       # 256 dv per core
QT = 256             # q tile
NQT = S // QT        # 8 q tiles
KB = 128             # k block
EPS = 1e-5
SCALE = 1.0 / np.sqrt(ND)

# iteration order: even tiles first so the even-parity AllToAll can fire at
# ~44% of the attention work and overlap the odd-tile compute.
ITERS = (0, 2, 4, 6, 1, 3, 5, 7)
# K/V 128-blocks projected at each iteration (front-loaded so tile t always
# has K/V blocks 0..2t+1 available).
KV_SCHED = {0: (0, 1), 2: (2, 3, 4, 5), 4: (6, 7, 8, 9), 6: (10, 11, 12, 13),
            1: (14, 15), 3: (), 5: (), 7: ()}

_cache = {}


def _install_ntff_shim():
    # antenv.axon_hooks is absent in this image; register the NTFF profile
    # hook so trace=True can capture HW exec time (harmless if unused).
    if "antenv.axon_hooks" in sys.modules:
        return
    mod = types.ModuleType("antenv.axon_hooks")
    mod._hook = None
    mod.set_axon_ntff_profile_hook = lambda h: setattr(mod, "_hook", h)
    mod.get_axon_ntff_profile_hook = lambda: mod._hook
    sys.modules["antenv.axon_hooks"] = mod
    try:
        import antenv

        antenv.axon_hooks = mod
        from trn_agent_boot.trn_boot import _ntff_profile_via_ctypes

        mod._hook = _ntff_profile_via_ctypes("/opt/axon/libaxon_pjrt.so")
    except Exception:
        pass


def _build():
    import concourse.bass as bass
    import concourse.mybir as mybir
    import concourse.tile as tile
    from concourse import bacc

    F32 = mybir.dt.float32
    F32R = mybir.dt.float32r
    BF16 = mybir.dt.bfloat16
    FP8 = mybir.dt.float8e4
    I32 = mybir.dt.int32
    ADD = mybir.AluOpType.add
    MUL = mybir.AluOpType.mult
    SUB = mybir.AluOpType.subtract
    SHR = mybir.AluOpType.logical_shift_right
    AF = mybir.ActivationFunctionType

    nc = bacc.Bacc("TRN2", target_bir_lowering=False, debug=False, num_devices=NC)

    def din(name, shape, dt=BF16):
        return nc.dram_tensor(name, shape, dt, kind="ExternalInput").ap()

    xtq = din("xtq", [D, S])
    xtk = din("xtk", [D, S])
    xtv = din("xtv", [D, S])
    wqT = din("wqT", [D, DVC])
    wkT = din("wkT", [D, DVC])
    wvT = din("wvT", [D, DVC])
    woT = din("woT", [D, D])
    smallc = din("smallc", [128, 288], F32)   # bq2|bk2|eps|pad|bv4x64(@16)
    gam_bc = din("gam_bc", [128, D], F32)
    bet_bc = din("bet_bc", [128, D], F32)
    resid = din("resid", [512, D], F32)       # x_q rows + bo (host pre-added)
    mo_in = din("mo", [128, 2 * QT])          # maskA|maskB (bf16)
    mo8_in = din("mo8", [128, 2 * QT], mybir.dt.float8e4)
    ones_r = din("ones_r", [1, 64], F32R)
    out_d = nc.dram_tensor("out", [512, D], F32, kind="ExternalOutput").ap()

    groups = [list(range(NC))]

    with nc.allow_low_precision(reason="f32r/bf16 matmul operand chain"), tile.TileContext(
        nc
    ) as tc:
        with (
            tc.tile_pool(name="const", bufs=1) as cpool,
            tc.tile_pool(name="res", bufs=1) as rpool,
            tc.tile_pool(name="xt", bufs=8) as xtpool,
            tc.tile_pool(name="pt", bufs=3) as ptpool,
            tc.tile_pool(name="dn", bufs=2) as dnpool,
            tc.tile_pool(name="gath", bufs=1) as gathpool,
            tc.tile_pool(name="ln", bufs=2) as lnpool,
            tc.tile_pool(name="ps_s", bufs=2, space="PSUM") as pss,
            tc.tile_pool(name="ps_ctx", bufs=2, space="PSUM") as psc,
            tc.tile_pool(name="ps_m", bufs=2, space="PSUM") as psm,
            tc.tile_pool(name="dram", bufs=1, space="DRAM") as dram,
        ):
            # ---- small constants + projection weights (needed first) ----
            smallc_sb = cpool.tile([128, 288], F32)
            mo_sb = cpool.tile([128, 2 * QT], BF16)
            mo8_sb = cpool.tile([128, 2 * QT], FP8)
            ones_sb = cpool.tile([1, 64], F32R)
            wq_sb = cpool.tile([128, 8, DVC], BF16)
            wk_sb = cpool.tile([128, 8, DVC], BF16)
            wv_sb = cpool.tile([128, 8, DVC], BF16)
            # spread startup loads across engine queues so issue overlaps
            nc.sync.dma_start(wk_sb[:], wkT.rearrange("(c p) n -> p c n", p=128))
            nc.scalar.dma_start(wq_sb[:], wqT.rearrange("(c p) n -> p c n", p=128))
            nc.gpsimd.dma_start(wv_sb[:], wvT.rearrange("(c p) n -> p c n", p=128))
            nc.sync.dma_start(smallc_sb[:], smallc)
            nc.sync.dma_start(ones_sb[:], ones_r)
            nc.sync.dma_start(mo_sb[:], mo_in)
            nc.sync.dma_start(mo8_sb[:], mo8_in)
            bq_sb = smallc_sb[:, 0:2]
            bk_sb = smallc_sb[:, 2:4]
            eps_sb = smallc_sb[:, 12:13]
            nbias_sb = smallc_sb[:, 13:14]  # -2.0 exp bias
            bv_sb = smallc_sb[:, 16:16 + DVC]   # bv broadcast (no ones col)
            mAB_sb = mo_sb[:, 0:2 * QT]

            # ---- resident activation tensors ----
            QT_sb = rpool.tile([128, 2, S], BF16)   # q^T: [dd(2x128), q]
            KT_sb = rpool.tile([128, 2, S], BF16)   # k^T: [dd(2x128), kpos]
            V_sb = rpool.tile([128, S // 128, HPC * (ND + 4)], FP8)
            ctx_sb = rpool.tile([128, 2, S], BF16)  # ctx^T: [dv(2x128), q]
            # ones columns of the V slots (denominator trick), set once;
            # slots are 68 wide (16B-aligned strides for dual-fp8 ldweights):
            # 64 data cols, a ones col, 3 zero pad cols.
            nc.gpsimd.memset(V_sb[:], 0.0)
            nc.gpsimd.memset(
                V_sb[:].rearrange("p c (h x) -> p c h x", x=ND + 4)[:, :, :, ND:ND + 1],
                1.0,
            )

            # ---- heavyweight phase-3 constants: loaded later (see below) --
            wo_sb = cpool.tile([128, 8, D], BF16)
            gam_sb = cpool.tile([128, D], F32)
            bet_sb = cpool.tile([128, D], F32)

            # ---- A2A buffers ----
            # Row ownership is interleaved at 128-row granularity across BOTH
            # batches: core j owns rows [512*(j//2)+128*(j%2), +128) of each
            # batch (even-tile set, parity 0) plus the same +256 (odd set).
            # Every A2A slot then carries real data -- no batch-dup zeros, no
            # receive-side select -- at half the previous payload.
            a2a_in = [
                dram.tile([NC, DVC, 128], BF16, name=f"a2a_in{i}") for i in range(2)
            ]
            a2a_out = [
                dram.tile([NC, DVC, 128], BF16, name=f"a2a_out{i}") for i in range(2)
            ]

            def proj_kq(w_sb, xt_d, b_sb, o_sb, c0):
                # project 256 source columns [c0, c0+256) into o_sb (K^T/Q^T)
                xts = xtpool.tile([128, 8, 256], BF16, tag="xt")
                nc.sync.dma_start(
                    xts[:],
                    xt_d.rearrange("(c p) n -> p c n", p=128)[:, :, c0:c0 + 256],
                )
                for m in range(2):
                    ps = psm.tile([128, 512], F32, tag="m")
                    for cc in range(8):
                        nc.tensor.matmul(
                            ps[:, 0:256],
                            lhsT=w_sb[:, cc, 128 * m:128 * m + 128],
                            rhs=xts[:, cc, :],
                            start=(cc == 0),
                            stop=(cc == 7),
                        )
                    nc.vector.tensor_scalar(
                        out=o_sb[:, m, c0:c0 + 256],
                        in0=ps[:, 0:256],
                        scalar1=b_sb[:, m:m + 1],
                        scalar2=None,
                        op0=ADD,
                    )

            def proj_v(c0):
                # project V for k rows [c0, c0+256) (two 128-blocks)
                xvs = xtpool.tile([128, 8, 256], BF16, tag="xt")
                nc.sync.dma_start(
                    xvs[:],
                    xtv.rearrange("(c p) n -> p c n", p=128)[:, :, c0:c0 + 256],
                )
                for r in range(2):
                    rc = c0 // 128 + r
                    ps = psm.tile([128, 512], F32, tag="m")
                    for cc in range(8):
                        nc.tensor.matmul(
                            ps[:, 0:DVC],
                            lhsT=xvs[:, cc, 128 * r:128 * r + 128],
                            rhs=wv_sb[:, cc, :],
                            start=(cc == 0),
                            stop=(cc == 7),
                        )
                    v_slot = V_sb[:, rc, :].rearrange("p (h x) -> p h x", x=ND + 4)[
                        :, :, 0:ND
                    ]
                    nc.vector.tensor_tensor(
                        out=v_slot,
                        in0=ps[:, 0:DVC].rearrange("p (h x) -> p h x", x=ND),
                        in1=bv_sb.rearrange("p (h x) -> p h x", x=ND),
                        op=ADD,
                    )

            def emit_pv(h, grp, pt, ctxps, t):
                co = 256 * (h % 2)
                ptv = pt.rearrange("p (b q) -> p b q", q=256)
                for idx, jp in enumerate(grp):
                    nc.tensor.matmul(
                        ctxps[0:ND + 4, co:co + 256],
                        lhsT=V_sb[:, 2 * jp:2 * jp + 2, (ND + 4) * h:(ND + 4) * (h + 1)],
                        rhs=ptv[:, 2 * idx:2 * idx + 2, :],
                        start=(jp == 0),
                        stop=(jp == t),
                        perf_mode=mybir.MatmulPerfMode.DoubleRow,
                        skip_group_check=True,
                    )

            # Deferred per-tile epilogue: the denominator broadcast matmul,
            # the normalize-divides, and the ship DMAs of tile t run during
            # iteration t+1 so the PE never waits on the reciprocal chain.
            def finish_tile(pend):
                t, ctxps_pair, dn, rcp = pend
                for pi in range(2):
                    bps = psm.tile([128, 512], F32, tag="m")
                    nc.tensor.matmul(
                        bps[0:64, :],
                        lhsT=ones_sb[0:1, :],
                        rhs=dn[0:1, 512 * pi:512 * pi + 512],
                        start=True,
                        stop=True,
                    )
                    nc.vector.tensor_copy(
                        rcp[:, 512 * pi:512 * pi + 512],
                        bps[0:64, :],
                    )
                for h in range(HPC):
                    po = 64 * (h % 2)
                    hc = h // 2
                    co = 256 * (h % 2)
                    nc.vector.tensor_tensor(
                        out=ctx_sb[po:po + 64, hc, QT * t:QT * t + QT],
                        in0=ctxps_pair[h // 2][0:64, co:co + 256],
                        in1=rcp[:, 512 * (h // 2) + co:512 * (h // 2) + co + 256],
                        op=MUL,
                    )
                ha = t % 2
                for hh in range(2):
                    dest = (t - ha) + hh
                    for m in range(2):
                        nc.sync.dma_start(
                            a2a_in[ha][dest, 128 * m:128 * m + 128, :],
                            ctx_sb[:, m, QT * t + 128 * hh:QT * t + 128 * hh + 128],
                        )
                if t == 6:
                    nc.gpsimd.collective_compute(
                        "AllToAll",
                        mybir.AluOpType.bypass,
                        replica_groups=groups,
                        ins=[a2a_in[0].opt()],
                        outs=[a2a_out[0].opt()],
                    )

            # ================= main loop =================
            pending = None
            for i, t in enumerate(ITERS):
                # ---- streamed projections for this iteration ----
                blocks = KV_SCHED[t]
                for p0 in range(0, len(blocks), 2):
                    proj_kq(wk_sb, xtk, bk_sb, KT_sb, blocks[p0] * 128)
                proj_kq(wq_sb, xtq, bq_sb, QT_sb, QT * t)
                for p0 in range(0, len(blocks), 2):
                    proj_v(blocks[p0] * 128)
                if pending is not None:
                    finish_tile(pending)
                    pending = None
                if i == 4:
                    # phase-3 constants: load mid-flight, off the hot window
                    nc.sync.dma_start(
                        wo_sb[:], woT.rearrange("(c p) n -> p c n", p=128)
                    )
                    nc.sync.dma_start(gam_sb[:], gam_bc)
                    nc.sync.dma_start(bet_sb[:], bet_bc)

                # ---- attention for q-tile t ----
                ctxps_pair = []
                for _pi in range(2):
                    cpt = psc.tile([128, 512], F32, tag="c")
                    ctxps_pair.append(cpt)
                for h in range(HPC):
                    po = 64 * (h % 2)
                    hc = h // 2
                    ctxps = ctxps_pair[h // 2]
                    q_rhs = QT_sb[po:po + 64, hc, QT * t:QT * t + QT]
                    jps = list(range(t + 1))
                    grps = [tuple(jps[k:k + 2]) for k in range(0, len(jps), 2)]
                    prev = None
                    for grp in grps:
                        w = 512 * len(grp)
                        sps = pss.tile([128, 1024], F32, tag="s")
                        for idx, jp in enumerate(grp):
                            for u in range(2):
                                nc.tensor.matmul(
                                    sps[:, 256 * (2 * idx + u):256 * (2 * idx + u) + 256],
                                    lhsT=KT_sb[
                                        po:po + 64,
                                        hc,
                                        128 * (2 * jp + u):128 * (2 * jp + u) + 128,
                                    ],
                                    rhs=q_rhs,
                                    start=True,
                                    stop=True,
                                )
                        pt = ptpool.tile([128, 1024], FP8, tag="pt")
                        # bias -2 keeps exp() under fp8e4 max; it cancels
                        # in softmax (the ones-column denominator sums the
                        # same fp8 values).
                        nc.scalar.activation(
                            pt[:, 0:w], sps[:, 0:w], AF.Exp, scale=SCALE, bias=nbias_sb
                        )
                        if t in grp:
                            do = 512 * grp.index(t)
                            nc.vector.tensor_tensor(
                                out=pt[:, do:do + 512],
                                in0=pt[:, do:do + 512],
                                in1=mo8_sb,
                                op=MUL,
                            )
                        if prev is not None:
                            emit_pv(h, prev[0], prev[1], ctxps, t)
                        prev = (grp, pt)
                    emit_pv(h, prev[0], prev[1], ctxps, t)

                # ---- denominator reciprocal chain (ACT/DVE, overlaps PV) ----
                dcp = dnpool.tile([1, 1024], F32, tag="dcp")
                dn0 = dnpool.tile([1, 1024], F32, tag="dn0")
                dn = dnpool.tile([1, 1024], F32R, tag="dn")
                rcp = dnpool.tile([64, 1024], F32, tag="rcp")
                for pi in range(2):
                    nc.scalar.activation(
                        dcp[0:1, 512 * pi:512 * pi + 512],
                        ctxps_pair[pi][64:65, 0:512],
                        AF.Copy,
                    )
                    nc.vector.reciprocal_approx_fast(
                        out=dn0[0:1, 512 * pi:512 * pi + 512],
                        in_=dcp[0:1, 512 * pi:512 * pi + 512],
                    )
                    nc.vector.tensor_copy(
                        dn[0:1, 512 * pi:512 * pi + 512],
                        dn0[0:1, 512 * pi:512 * pi + 512],
                    )
                pending = (t, ctxps_pair, dn, rcp)

            # tile 7's epilogue + collective #1 dispatch FIRST, so the
            # collective's flight overlaps the ha=0 output projection.
            finish_tile(pending)
            nc.gpsimd.collective_compute(
                "AllToAll",
                mybir.AluOpType.bypass,
                replica_groups=groups,
                ins=[a2a_in[1].opt()],
                outs=[a2a_out[1].opt()],
            )

            # ---- phase 3: gather + output projection + residual + LN ----
            half = 1.5
            for ha in range(2):
                gath = gathpool.tile([128, 2, 8, 128], BF16, tag=f"gath{ha}")
                for bb in range(2):
                    for gp in range(4):
                        for m in range(2):
                            nc.sync.dma_start(
                                gath[:, bb, 2 * gp + m, :],
                                a2a_out[ha][4 * bb + gp, 128 * m:128 * m + 128, :],
                            )
                for rc in range(2):
                    R = 2 * ha + rc  # local 128-row chunk index (batch rc)
                    y_sb = lnpool.tile([128, D], F32, tag="y")
                    res_sb = lnpool.tile([128, D], F32, tag="res")
                    nc.sync.dma_start(res_sb[:], resid[128 * R:128 * R + 128, :])
                    for n in range(2):
                        ps = psm.tile([128, 512], F32, tag="m")
                        for d2 in range(8):
                            nc.tensor.matmul(
                                ps[:],
                                lhsT=gath[:, rc, d2, :],
                                rhs=wo_sb[:, d2, 512 * n:512 * n + 512],
                                start=(d2 == 0),
                                stop=(d2 == 7),
                            )
                        nc.vector.tensor_tensor(
                            out=y_sb[:, 512 * n:512 * n + 512],
                            in0=ps[:],
                            in1=res_sb[:, 512 * n:512 * n + 512],
                            op=ADD,
                        )
                    # LayerNorm over D: bn_stats mean/var + DVE rsqrt bit-trick
                    st = lnpool.tile([128, 16], F32, tag="st")
                    sti = lnpool.tile([128, 2], I32, tag="sti")
                    nc.vector.bn_stats(st[:, 0:6], y_sb[:, 0:512])
                    nc.vector.bn_stats(st[:, 6:12], y_sb[:, 512:1024])
                    nc.vector.bn_aggr(st[:, 12:14], st[:, 0:12])
                    mu = st[:, 12:13]
                    # v = var + eps; y0 = bitcast(0x5f3759df - (v_int >> 1))
                    nc.vector.tensor_tensor(
                        out=st[:, 14:15], in0=st[:, 13:14], in1=eps_sb, op=ADD
                    )
                    v = st[:, 14:15]
                    nc.vector.tensor_scalar(
                        out=sti[:, 0:1], in0=v.bitcast(I32), scalar1=1,
                        scalar2=None, op0=SHR,
                    )
                    nc.vector.tensor_scalar(
                        out=sti[:, 1:2], in0=sti[:, 0:1], scalar1=-1,
                        scalar2=0x5F3759DF, op0=MUL, op1=ADD,
                    )
                    y0 = sti[:, 1:2].bitcast(F32)
                    # h2 = -0.5 v ; two Newton steps: y <- y*(1.5 + h2*y*y)
                    nc.vector.tensor_scalar(
                        out=st[:, 15:16], in0=v, scalar1=-0.5, scalar2=None, op0=MUL
                    )
                    h2 = st[:, 15:16]
                    nc.vector.tensor_tensor(out=st[:, 0:1], in0=y0, in1=y0, op=MUL)
                    nc.vector.tensor_scalar(
                        out=st[:, 1:2], in0=st[:, 0:1], scalar1=h2, scalar2=half,
                        op0=MUL, op1=ADD,
                    )
                    nc.vector.tensor_tensor(out=st[:, 2:3], in0=y0, in1=st[:, 1:2], op=MUL)
                    nc.vector.tensor_tensor(
                        out=st[:, 3:4], in0=st[:, 2:3], in1=st[:, 2:3], op=MUL
                    )
                    nc.vector.tensor_scalar(
                        out=st[:, 4:5], in0=st[:, 3:4], scalar1=h2, scalar2=half,
                        op0=MUL, op1=ADD,
                    )
                    nc.vector.tensor_tensor(out=st[:, 5:6], in0=st[:, 2:3], in1=st[:, 4:5], op=MUL)
                    rstd = st[:, 5:6]
                    # yc = (y - mu) * rstd ; out = yc*gamma + beta
                    yc = lnpool.tile([128, D], F32, tag="yc")
                    nc.vector.tensor_scalar(
                        out=yc[:], in0=y_sb[:],
                        scalar1=mu, scalar2=rstd, op0=SUB, op1=MUL,
                    )
                    nc.vector.tensor_tensor(out=yc[:], in0=yc[:], in1=gam_sb[:], op=MUL)
                    nc.vector.tensor_tensor(out=yc[:], in0=yc[:], in1=bet_sb[:], op=ADD)
                    nc.sync.dma_start(out_d[128 * R:128 * R + 128, :], yc[:])

    nc.compile()
    return nc


def _prep_inputs(x_q, x_k, x_v, mask, Wq, bq, Wk, bk, Wv, bv, Wo, bo, gamma, beta):
    import ml_dtypes

    f = np.float32
    bf = ml_dtypes.bfloat16
    maskA = np.zeros((KB, QT), f)
    maskB = np.zeros((KB, QT), f)
    for i in range(KB):
        maskA[i, i:] = 1.0
        if i + 128 < QT:
            maskB[i, i + 128:] = 1.0
    mo = np.concatenate([maskA, maskB], axis=1).astype(bf)
    mo8 = np.concatenate([maskA, maskB], axis=1).astype(ml_dtypes.float8_e4m3)
    in_maps = []
    for c in range(NC):
        b, g = c // 4, c % 4
        dv = slice(DVC * g, DVC * (g + 1))
        # interleaved cross-batch row ownership (see A2A comment in _build)
        re = 512 * (c // 2) + 128 * (c % 2)
        ro = re + 256
        smallc = np.zeros((128, 288), f)
        smallc[:, 0:2] = bq[dv].astype(f).reshape(2, 128).T
        smallc[:, 2:4] = bk[dv].astype(f).reshape(2, 128).T
        smallc[:, 4] = 1.0 - b
        smallc[:, 5] = float(b)
        smallc[:, 12] = EPS
        smallc[:, 13] = -2.0
        smallc[:, 16:16 + DVC] = np.broadcast_to(bv[dv].astype(f), (128, DVC))
        in_maps.append(
            {
                "xtq": np.ascontiguousarray(x_q[b].T.astype(bf)),
                "xtk": np.ascontiguousarray(x_k[b].T.astype(bf)),
                "xtv": np.ascontiguousarray(x_v[b].T.astype(bf)),
                "wqT": np.ascontiguousarray(Wq[dv, :].T.astype(bf)),
                "wkT": np.ascontiguousarray(Wk[dv, :].T.astype(bf)),
                "wvT": np.ascontiguousarray(Wv[dv, :].T.astype(bf)),
                "woT": np.ascontiguousarray(Wo.T.astype(bf)),
                "smallc": smallc,
                "gam_bc": np.broadcast_to(gamma.astype(f), (128, D)).copy(),
                "bet_bc": np.broadcast_to(beta.astype(f), (128, D)).copy(),
                "resid": np.ascontiguousarray(
                    np.concatenate(
                        [
                            x_q[0, re:re + 128, :],
                            x_q[1, re:re + 128, :],
                            x_q[0, ro:ro + 128, :],
                            x_q[1, ro:ro + 128, :],
                        ]
                    ).astype(f)
                    + bo.astype(f)
                ),
                "mo": mo,
                "mo8": mo8,
                "ones_r": np.ones((1, 64), f),
            }
        )
    return in_maps


def kernel(x_q, x_k, x_v, mask, Wq, bq, Wk, bk, Wv, bv, Wo, bo, gamma, beta):
    _install_ntff_shim()
    from concourse.bass_utils import run_bass_kernel_spmd

    x_q, x_k, x_v = np.asarray(x_q), np.asarray(x_k), np.asarray(x_v)
    mask = np.asarray(mask)
    # this kernel implements causal attention structurally; verify the mask
    causal = np.tril(np.ones((S, S), mask.dtype))
    assert np.array_equal(mask.reshape(S, S), causal), "kernel specialized for causal mask"

    if "nc" not in _cache:
        _cache["nc"] = _build()
    nc = _cache["nc"]

    in_maps = _prep_inputs(
        x_q, x_k, x_v, mask,
        np.asarray(Wq), np.asarray(bq), np.asarray(Wk), np.asarray(bk),
        np.asarray(Wv), np.asarray(bv), np.asarray(Wo), np.asarray(bo),
        np.asarray(gamma), np.asarray(beta),
    )
    res = run_bass_kernel_spmd(nc, in_maps, list(range(NC)))
    _cache["last_results"] = res

    out = np.empty((B, S, D), np.float32)
    for c in range(NC):
        re = 512 * (c // 2) + 128 * (c % 2)
        ro = re + 256
        r = res.results[c]["out"]
        out[0, re:re + 128, :] = r[0:128]
        out[1, re:re + 128, :] = r[128:256]
        out[0, ro:ro + 128, :] = r[256:384]
        out[1, ro:ro + 128, :] = r[384:512]
    return out


# revision 58
# speedup vs baseline: 1.3052x; 1.0477x over previous
# Trainium2 Bass kernel for nn_MultiHeadAttention_87024627352037.
#
# Full module: y = LayerNorm(x_q + (softmax(mask(QK^T/sqrt(nd))) V) Wo^T + bo)
# with Q/K/V projections of x_q/x_k/x_v. Shapes: B=2, S=2048, D=1024, H=16.
#
# Sharding (8 cores): core c = (batch b=c//4, head-quad g=c%4).
# Each core projects Q/K/V for its 4 heads (dv=256) over its batch and runs
# causal attention in a fully transposed layout (scoresT = K_T^T Q_T, no
# max-subtraction -- scores are O(1); softmax denominator via a ones-column
# in the PV matmul). Projections are streamed and interleaved with the
# attention q-tiles (processed 0,2,4,6,1,3,5,7) so the PE ramps early and
# stays busy. The ACT engine runs *only* Exp (no table reloads): the
# denominator reciprocal is computed on DVE and broadcast across partitions
# with a tiny f32r matmul; LayerNorm's rsqrt uses a DVE bit-trick + Newton
# steps. PSUM->SBUF fixups ride on the idle GPSIMD engine. A per-batch
# AllToAll (groups of 4) re-shards ctx from head-sharding to row-sharding;
# each core computes output projection + residual + LayerNorm for its 512
# rows. The host only slices, transposes, and concatenates numpy arrays.
import os
import sys
import types

import numpy as np

B, S, D, H = 2, 2048, 1024, 16
ND = D // H          # 64
NC = 8               # cores
HPC = H // 4         # 4 heads per core
DVC = HPC * ND       # 256 dv per core
QT = 256             # q tile
NQT = S // QT        # 8 q tiles
KB = 128             # k block
EPS = 1e-5
SCALE = 1.0 / np.sqrt(ND)

# iteration order: even tiles first so the even-parity AllToAll can fire at
# ~44% of the attention work and overlap the odd-tile compute.
ITERS = (0, 2, 4, 6, 1, 3, 5, 7)
# K/V 128-blocks projected at each iteration (front-loaded so tile t always
# has K/V blocks 0..2t+1 available).
KV_SCHED = {0: (0, 1), 2: (2, 3, 4, 5), 4: (6, 7, 8, 9), 6: (10, 11, 12, 13),
            1: (14, 15), 3: (), 5: (), 7: ()}

_cache = {}


def _install_ntff_shim():
    # antenv.axon_hooks is absent in this image; register the NTFF profile
    # hook so trace=True can capture HW exec time (harmless if unused).
    if "antenv.axon_hooks" in sys.modules:
        return
    mod = types.ModuleType("antenv.axon_hooks")
    mod._hook = None
    mod.set_axon_ntff_profile_hook = lambda h: setattr(mod, "_hook", h)
    mod.get_axon_ntff_profile_hook = lambda: mod._hook
    sys.modules["antenv.axon_hooks"] = mod
    try:
        import antenv

        antenv.axon_hooks = mod
        from trn_agent_boot.trn_boot import _ntff_profile_via_ctypes

        mod._hook = _ntff_profile_via_ctypes("/opt/axon/libaxon_pjrt.so")
    except Exception:
        pass


def _build():
    import concourse.bass as bass
    import concourse.mybir as mybir
    import concourse.tile as tile
    from concourse import bacc

    F32 = mybir.dt.float32
    F32R = mybir.dt.float32r
    BF16 = mybir.dt.bfloat16
    FP8 = mybir.dt.float8e4
    I32 = mybir.dt.int32
    ADD = mybir.AluOpType.add
    MUL = mybir.AluOpType.mult
    SUB = mybir.AluOpType.subtract
    SHR = mybir.AluOpType.logical_shift_right
    AF = mybir.ActivationFunctionType

    nc = bacc.Bacc("TRN2", target_bir_lowering=False, debug=False, num_devices=NC)

    def din(name, shape, dt=BF16):
        return nc.dram_tensor(name, shape, dt, kind="ExternalInput").ap()

    xtq = din("xtq", [D, S])
    xtk = din("xtk", [D, S])
    xtv = din("xtv", [D, S])
    wqT = din("wqT", [D, DVC])
    wkT = din("wkT", [D, DVC])
    wvT = din("wvT", [D, DVC])
    woT = din("woT", [D, D])
    smallc = din("smallc", [128, 288], F32)   # bq2|bk2|eps|pad|bv4x64(@16)
    gam_bc = din("gam_bc", [128, D], F32)
    bet_bc = din("bet_bc", [128, D], F32)
    resid = din("resid", [512, D], F32)       # x_q rows + bo (host pre-added)
    mo_in = din("mo", [128, 2 * QT])          # maskA|maskB (bf16)
    mo8_in = din("mo8", [128, 2 * QT], mybir.dt.float8e4)
    ones_r = din("ones_r", [1, 64], F32R)
    out_d = nc.dram_tensor("out", [512, D], F32, kind="ExternalOutput").ap()

    groups = [list(range(NC))]

    with nc.allow_low_precision(reason="f32r/bf16 matmul operand chain"), tile.TileContext(
        nc
    ) as tc:
        with (
            tc.tile_pool(name="const", bufs=1) as cpool,
            tc.tile_pool(name="res", bufs=1) as rpool,
            tc.tile_pool(name="xt", bufs=8) as xtpool,
            tc.tile_pool(name="pt", bufs=3) as ptpool,
            tc.tile_pool(name="dn", bufs=2) as dnpool,
            tc.tile_pool(name="gath", bufs=1) as gathpool,
            tc.tile_pool(name="ln", bufs=2) as lnpool,
            tc.tile_pool(name="ps_s", bufs=2, space="PSUM") as pss,
            tc.tile_pool(name="ps_ctx", bufs=2, space="PSUM") as psc,
            tc.tile_pool(name="ps_m", bufs=2, space="PSUM") as psm,
            tc.tile_pool(name="dram", bufs=1, space="DRAM") as dram,
        ):
            # ---- small constants + projection weights (needed first) ----
            smallc_sb = cpool.tile([128, 288], F32)
            mo_sb = cpool.tile([128, 2 * QT], BF16)
            mo8_sb = cpool.tile([128, 2 * QT], FP8)
            ones_sb = cpool.tile([1, 64], F32R)
            wq_sb = cpool.tile([128, 8, DVC], BF16)
            wk_sb = cpool.tile([128, 8, DVC], BF16)
            wv_sb = cpool.tile([128, 8, DVC], BF16)
            # spread startup loads across engine queues so issue overlaps
            nc.sync.dma_start(wk_sb[:], wkT.rearrange("(c p) n -> p c n", p=128))
            nc.scalar.dma_start(wq_sb[:], wqT.rearrange("(c p) n -> p c n", p=128))
            nc.gpsimd.dma_start(wv_sb[:], wvT.rearrange("(c p) n -> p c n", p=128))
            nc.scalar.dma_start(smallc_sb[:], smallc)
            nc.gpsimd.dma_start(ones_sb[:], ones_r)
            nc.scalar.dma_start(mo_sb[:], mo_in)
            nc.gpsimd.dma_start(mo8_sb[:], mo8_in)
            bq_sb = smallc_sb[:, 0:2]
            bk_sb = smallc_sb[:, 2:4]
            eps_sb = smallc_sb[:, 12:13]
            nbias_sb = smallc_sb[:, 13:14]  # -2.0 exp bias
            bv_sb = smallc_sb[:, 16:16 + DVC]   # bv broadcast (no ones col)
            mAB_sb = mo_sb[:, 0:2 * QT]

            # ---- resident activation tensors ----
            QT_sb = rpool.tile([128, 2, S], BF16)   # q^T: [dd(2x128), q]
            KT_sb = rpool.tile([128, 2, S], BF16)   # k^T: [dd(2x128), kpos]
            V_sb = rpool.tile([128, S // 128, HPC * (ND + 4)], FP8)
            ctx_sb = rpool.tile([128, 2, S], BF16)  # ctx^T: [dv(2x128), q]
            # ones columns of the V slots (denominator trick), set once;
            # slots are 68 wide (16B-aligned strides for dual-fp8 ldweights):
            # 64 data cols, a ones col, 3 zero pad cols.
            nc.gpsimd.memset(V_sb[:], 0.0)
            nc.gpsimd.memset(
                V_sb[:].rearrange("p c (h x) -> p c h x", x=ND + 4)[:, :, :, ND:ND + 1],
                1.0,
            )

            # ---- heavyweight phase-3 constants: loaded later (see below) --
            wo_sb = cpool.tile([128, 8, D], BF16)
            gam_sb = cpool.tile([128, D], F32)
            bet_sb = cpool.tile([128, D], F32)

            # ---- A2A buffers ----
            # Row ownership is interleaved at 128-row granularity across BOTH
            # batches: core j owns rows [512*(j//2)+128*(j%2), +128) of each
            # batch (even-tile set, parity 0) plus the same +256 (odd set).
            # Every A2A slot then carries real data -- no batch-dup zeros, no
            # receive-side select -- at half the previous payload.
            a2a_in = [
                dram.tile([NC, DVC, 128], BF16, name=f"a2a_in{i}") for i in range(2)
            ]
            a2a_out = [
                dram.tile([NC, DVC, 128], BF16, name=f"a2a_out{i}") for i in range(2)
            ]

            def proj_kq(w_sb, xt_d, b_sb, o_sb, c0):
                # project 256 source columns [c0, c0+256) into o_sb (K^T/Q^T)
                xts = xtpool.tile([128, 8, 256], BF16, tag="xt")
                nc.sync.dma_start(
                    xts[:],
                    xt_d.rearrange("(c p) n -> p c n", p=128)[:, :, c0:c0 + 256],
                )
                for m in range(2):
                    ps = psm.tile([128, 512], F32, tag="m")
                    for cc in range(8):
                        nc.tensor.matmul(
                            ps[:, 0:256],
                            lhsT=w_sb[:, cc, 128 * m:128 * m + 128],
                            rhs=xts[:, cc, :],
                            start=(cc == 0),
                            stop=(cc == 7),
                        )
                    nc.vector.tensor_scalar(
                        out=o_sb[:, m, c0:c0 + 256],
                        in0=ps[:, 0:256],
                        scalar1=b_sb[:, m:m + 1],
                        scalar2=None,
                        op0=ADD,
                    )

            def proj_v(c0):
                # project V for k rows [c0, c0+256) (two 128-blocks)
                xvs = xtpool.tile([128, 8, 256], BF16, tag="xt")
                nc.sync.dma_start(
                    xvs[:],
                    xtv.rearrange("(c p) n -> p c n", p=128)[:, :, c0:c0 + 256],
                )
                for r in range(2):
                    rc = c0 // 128 + r
                    ps = psm.tile([128, 512], F32, tag="m")
                    for cc in range(8):
                        nc.tensor.matmul(
                            ps[:, 0:DVC],
                            lhsT=xvs[:, cc, 128 * r:128 * r + 128],
                            rhs=wv_sb[:, cc, :],
                            start=(cc == 0),
                            stop=(cc == 7),
                        )
                    v_slot = V_sb[:, rc, :].rearrange("p (h x) -> p h x", x=ND + 4)[
                        :, :, 0:ND
                    ]
                    nc.vector.tensor_tensor(
                        out=v_slot,
                        in0=ps[:, 0:DVC].rearrange("p (h x) -> p h x", x=ND),
                        in1=bv_sb.rearrange("p (h x) -> p h x", x=ND),
                        op=ADD,
                    )

            def emit_pv(h, grp, pt, ctxps, t):
                co = 256 * (h % 2)
                ptv = pt.rearrange("p (b q) -> p b q", q=256)
                for idx, jp in enumerate(grp):
                    nc.tensor.matmul(
                        ctxps[0:ND + 4, co:co + 256],
                        lhsT=V_sb[:, 2 * jp:2 * jp + 2, (ND + 4) * h:(ND + 4) * (h + 1)],
                        rhs=ptv[:, 2 * idx:2 * idx + 2, :],
                        start=(jp == 0),
                        stop=(jp == t),
                        perf_mode=mybir.MatmulPerfMode.DoubleRow,
                        skip_group_check=True,
                    )

            # Deferred per-tile epilogue: the denominator broadcast matmul,
            # the normalize-divides, and the ship DMAs of tile t run during
            # iteration t+1 so the PE never waits on the reciprocal chain.
            def finish_tile(pend):
                t, ctxps_pair, dn, rcp = pend
                for pi in range(2):
                    bps = psm.tile([128, 512], F32, tag="m")
                    nc.tensor.matmul(
                        bps[0:64, :],
                        lhsT=ones_sb[0:1, :],
                        rhs=dn[0:1, 512 * pi:512 * pi + 512],
                        start=True,
                        stop=True,
                    )
                    nc.vector.tensor_copy(
                        rcp[:, 512 * pi:512 * pi + 512],
                        bps[0:64, :],
                    )
                for h in range(HPC):
                    po = 64 * (h % 2)
                    hc = h // 2
                    co = 256 * (h % 2)
                    nc.vector.tensor_tensor(
                        out=ctx_sb[po:po + 64, hc, QT * t:QT * t + QT],
                        in0=ctxps_pair[h // 2][0:64, co:co + 256],
                        in1=rcp[:, 512 * (h // 2) + co:512 * (h // 2) + co + 256],
                        op=MUL,
                    )
                ha = t % 2
                for hh in range(2):
                    dest = (t - ha) + hh
                    for m in range(2):
                        nc.sync.dma_start(
                            a2a_in[ha][dest, 128 * m:128 * m + 128, :],
                            ctx_sb[:, m, QT * t + 128 * hh:QT * t + 128 * hh + 128],
                        )
                if t == 6:
                    nc.gpsimd.collective_compute(
                        "AllToAll",
                        mybir.AluOpType.bypass,
                        replica_groups=groups,
                        ins=[a2a_in[0].opt()],
                        outs=[a2a_out[0].opt()],
                    )

            # ================= main loop =================
            pending = None
            for i, t in enumerate(ITERS):
                # ---- streamed projections for this iteration ----
                blocks = KV_SCHED[t]
                for p0 in range(0, len(blocks), 2):
                    proj_kq(wk_sb, xtk, bk_sb, KT_sb, blocks[p0] * 128)
                proj_kq(wq_sb, xtq, bq_sb, QT_sb, QT * t)
                for p0 in range(0, len(blocks), 2):
                    proj_v(blocks[p0] * 128)
                if pending is not None:
                    finish_tile(pending)
                    pending = None
                if i == 4:
                    # phase-3 constants: load mid-flight, off the hot window
                    nc.sync.dma_start(
                        wo_sb[:], woT.rearrange("(c p) n -> p c n", p=128)
                    )
                    nc.sync.dma_start(gam_sb[:], gam_bc)
                    nc.sync.dma_start(bet_sb[:], bet_bc)

                # ---- attention for q-tile t ----
                ctxps_pair = []
                for _pi in range(2):
                    cpt = psc.tile([128, 512], F32, tag="c")
                    ctxps_pair.append(cpt)
                for h in range(HPC):
                    po = 64 * (h % 2)
                    hc = h // 2
                    ctxps = ctxps_pair[h // 2]
                    q_rhs = QT_sb[po:po + 64, hc, QT * t:QT * t + QT]
                    jps = list(range(t + 1))
                    grps = [tuple(jps[k:k + 2]) for k in range(0, len(jps), 2)]
                    prev = None
                    for grp in grps:
                        w = 512 * len(grp)
                        sps = pss.tile([128, 1024], F32, tag="s")
                        for idx, jp in enumerate(grp):
                            for u in range(2):
                                nc.tensor.matmul(
                                    sps[:, 256 * (2 * idx + u):256 * (2 * idx + u) + 256],
                                    lhsT=KT_sb[
                                        po:po + 64,
                                        hc,
                                        128 * (2 * jp + u):128 * (2 * jp + u) + 128,
                                    ],
                                    rhs=q_rhs,
                                    start=True,
                                    stop=True,
                                )
                        pt = ptpool.tile([128, 1024], FP8, tag="pt")
                        # bias -2 keeps exp() under fp8e4 max; it cancels
                        # in softmax (the ones-column denominator sums the
                        # same fp8 values).
                        nc.scalar.activation(
                            pt[:, 0:w], sps[:, 0:w], AF.Exp, scale=SCALE, bias=nbias_sb
                        )
                        if t in grp:
                            do = 512 * grp.index(t)
                            nc.vector.tensor_tensor(
                                out=pt[:, do:do + 512],
                                in0=pt[:, do:do + 512],
                                in1=mo8_sb,
                                op=MUL,
                            )
                        if prev is not None:
                            emit_pv(h, prev[0], prev[1], ctxps, t)
                        prev = (grp, pt)
                    emit_pv(h, prev[0], prev[1], ctxps, t)

                # ---- denominator reciprocal chain (ACT/DVE, overlaps PV) ----
                dcp = dnpool.tile([1, 1024], F32, tag="dcp")
                dn0 = dnpool.tile([1, 1024], F32, tag="dn0")
                dn = dnpool.tile([1, 1024], F32R, tag="dn")
                rcp = dnpool.tile([64, 1024], F32, tag="rcp")
                for pi in range(2):
                    nc.scalar.activation(
                        dcp[0:1, 512 * pi:512 * pi + 512],
                        ctxps_pair[pi][64:65, 0:512],
                        AF.Copy,
                    )
                    nc.vector.reciprocal_approx_fast(
                        out=dn0[0:1, 512 * pi:512 * pi + 512],
                        in_=dcp[0:1, 512 * pi:512 * pi + 512],
                    )
                    nc.vector.tensor_copy(
                        dn[0:1, 512 * pi:512 * pi + 512],
                        dn0[0:1, 512 * pi:512 * pi + 512],
                    )
                pending = (t, ctxps_pair, dn, rcp)

            # tile 7's epilogue + collective #1 dispatch FIRST, so the
            # collective's flight overlaps the ha=0 output projection.
            finish_tile(pending)
            nc.gpsimd.collective_compute(
                "AllToAll",
                mybir.AluOpType.bypass,
                replica_groups=groups,
                ins=[a2a_in[1].opt()],
                outs=[a2a_out[1].opt()],
            )

            # ---- phase 3: gather + output projection + residual + LN ----
            half = 1.5
            for ha in range(2):
                gath = gathpool.tile([128, 2, 8, 128], BF16, tag=f"gath{ha}")
                for bb in range(2):
                    for gp in range(4):
                        for m in range(2):
                            nc.sync.dma_start(
                                gath[:, bb, 2 * gp + m, :],
                                a2a_out[ha][4 * bb + gp, 128 * m:128 * m + 128, :],
                            )
                for rc in range(2):
                    R = 2 * ha + rc  # local 128-row chunk index (batch rc)
                    y_sb = lnpool.tile([128, D], F32, tag="y")
                    res_sb = lnpool.tile([128, D], F32, tag="res")
                    nc.sync.dma_start(res_sb[:], resid[128 * R:128 * R + 128, :])
                    for n in range(2):
                        ps = psm.tile([128, 512], F32, tag="m")
                        for d2 in range(8):
                            nc.tensor.matmul(
                                ps[:],
                                lhsT=gath[:, rc, d2, :],
                                rhs=wo_sb[:, d2, 512 * n:512 * n + 512],
                                start=(d2 == 0),
                                stop=(d2 == 7),
                            )
                        nc.vector.tensor_tensor(
                            out=y_sb[:, 512 * n:512 * n + 512],
                            in0=ps[:],
                            in1=res_sb[:, 512 * n:512 * n + 512],
                            op=ADD,
                        )
                    # LayerNorm over D: bn_stats mean/var + DVE rsqrt bit-trick
                    st = lnpool.tile([128, 16], F32, tag="st")
                    sti = lnpool.tile([128, 2], I32, tag="sti")
                    nc.vector.bn_stats(st[:, 0:6], y_sb[:, 0:512])
                    nc.vector.bn_stats(st[:, 6:12], y_sb[:, 512:1024])
                    nc.vector.bn_aggr(st[:, 12:14], st[:, 0:12])
                    mu = st[:, 12:13]
                    # v = var + eps; y0 = bitcast(0x5f3759df - (v_int >> 1))
                    nc.vector.tensor_tensor(
                        out=st[:, 14:15], in0=st[:, 13:14], in1=eps_sb, op=ADD
                    )
                    v = st[:, 14:15]
                    nc.vector.tensor_scalar(
                        out=sti[:, 0:1], in0=v.bitcast(I32), scalar1=1,
                        scalar2=None, op0=SHR,
                    )
                    nc.vector.tensor_scalar(
                        out=sti[:, 1:2], in0=sti[:, 0:1], scalar1=-1,
                        scalar2=0x5F3759DF, op0=MUL, op1=ADD,
                    )
                    y0 = sti[:, 1:2].bitcast(F32)
                    # h2 = -0.5 v ; two Newton steps: y <- y*(1.5 + h2*y*y)
                    nc.vector.tensor_scalar(
                        out=st[:, 15:16], in0=v, scalar1=-0.5, scalar2=None, op0=MUL
                    )
                    h2 = st[:, 15:16]
                    nc.vector.tensor_tensor(out=st[:, 0:1], in0=y0, in1=y0, op=MUL)
                    nc.vector.tensor_scalar(
                        out=st[:, 1:2], in0=st[:, 0:1], scalar1=h2, scalar2=half,
                        op0=MUL, op1=ADD,
                    )
                    nc.vector.tensor_tensor(out=st[:, 2:3], in0=y0, in1=st[:, 1:2], op=MUL)
                    nc.vector.tensor_tensor(
                        out=st[:, 3:4], in0=st[:, 2:3], in1=st[:, 2:3], op=MUL
                    )
                    nc.vector.tensor_scalar(
                        out=st[:, 4:5], in0=st[:, 3:4], scalar1=h2, scalar2=half,
                        op0=MUL, op1=ADD,
                    )
                    nc.vector.tensor_tensor(out=st[:, 5:6], in0=st[:, 2:3], in1=st[:, 4:5], op=MUL)
                    rstd = st[:, 5:6]
                    # yc = (y - mu) * rstd ; out = yc*gamma + beta
                    yc = lnpool.tile([128, D], F32, tag="yc")
                    nc.vector.tensor_scalar(
                        out=yc[:], in0=y_sb[:],
                        scalar1=mu, scalar2=rstd, op0=SUB, op1=MUL,
                    )
                    nc.vector.tensor_tensor(out=yc[:], in0=yc[:], in1=gam_sb[:], op=MUL)
                    nc.vector.tensor_tensor(out=yc[:], in0=yc[:], in1=bet_sb[:], op=ADD)
                    nc.sync.dma_start(out_d[128 * R:128 * R + 128, :], yc[:])

    nc.compile()
    return nc


def _prep_inputs(x_q, x_k, x_v, mask, Wq, bq, Wk, bk, Wv, bv, Wo, bo, gamma, beta):
    import ml_dtypes

    f = np.float32
    bf = ml_dtypes.bfloat16
    maskA = np.zeros((KB, QT), f)
    maskB = np.zeros((KB, QT), f)
    for i in range(KB):
        maskA[i, i:] = 1.0
        if i + 128 < QT:
            maskB[i, i + 128:] = 1.0
    mo = np.concatenate([maskA, maskB], axis=1).astype(bf)
    mo8 = np.concatenate([maskA, maskB], axis=1).astype(ml_dtypes.float8_e4m3)
    in_maps = []
    for c in range(NC):
        b, g = c // 4, c % 4
        dv = slice(DVC * g, DVC * (g + 1))
        # interleaved cross-batch row ownership (see A2A comment in _build)
        re = 512 * (c // 2) + 128 * (c % 2)
        ro = re + 256
        smallc = np.zeros((128, 288), f)
        smallc[:, 0:2] = bq[dv].astype(f).reshape(2, 128).T
        smallc[:, 2:4] = bk[dv].astype(f).reshape(2, 128).T
        smallc[:, 4] = 1.0 - b
        smallc[:, 5] = float(b)
        smallc[:, 12] = EPS
        smallc[:, 13] = -2.0
        smallc[:, 16:16 + DVC] = np.broadcast_to(bv[dv].astype(f), (128, DVC))
        in_maps.append(
            {
                "xtq": np.ascontiguousarray(x_q[b].T.astype(bf)),
                "xtk": np.ascontiguousarray(x_k[b].T.astype(bf)),
                "xtv": np.ascontiguousarray(x_v[b].T.astype(bf)),
                "wqT": np.ascontiguousarray(Wq[dv, :].T.astype(bf)),
                "wkT": np.ascontiguousarray(Wk[dv, :].T.astype(bf)),
                "wvT": np.ascontiguousarray(Wv[dv, :].T.astype(bf)),
                "woT": np.ascontiguousarray(Wo.T.astype(bf)),
                "smallc": smallc,
                "gam_bc": np.broadcast_to(gamma.astype(f), (128, D)).copy(),
                "bet_bc": np.broadcast_to(beta.astype(f), (128, D)).copy(),
                "resid": np.ascontiguousarray(
                    np.concatenate(
                        [
                            x_q[0, re:re + 128, :],
                            x_q[1, re:re + 128, :],
                            x_q[0, ro:ro + 128, :],
                            x_q[1, ro:ro + 128, :],
                        ]
                    ).astype(f)
                    + bo.astype(f)
                ),
                "mo": mo,
                "mo8": mo8,
                "ones_r": np.ones((1, 64), f),
            }
        )
    return in_maps


def kernel(x_q, x_k, x_v, mask, Wq, bq, Wk, bk, Wv, bv, Wo, bo, gamma, beta):
    _install_ntff_shim()
    from concourse.bass_utils import run_bass_kernel_spmd

    x_q, x_k, x_v = np.asarray(x_q), np.asarray(x_k), np.asarray(x_v)
    mask = np.asarray(mask)
    # this kernel implements causal attention structurally; verify the mask
    causal = np.tril(np.ones((S, S), mask.dtype))
    assert np.array_equal(mask.reshape(S, S), causal), "kernel specialized for causal mask"

    if "nc" not in _cache:
        _cache["nc"] = _build()
    nc = _cache["nc"]

    in_maps = _prep_inputs(
        x_q, x_k, x_v, mask,
        np.asarray(Wq), np.asarray(bq), np.asarray(Wk), np.asarray(bk),
        np.asarray(Wv), np.asarray(bv), np.asarray(Wo), np.asarray(bo),
        np.asarray(gamma), np.asarray(beta),
    )
    res = run_bass_kernel_spmd(nc, in_maps, list(range(NC)))
    _cache["last_results"] = res

    out = np.empty((B, S, D), np.float32)
    for c in range(NC):
        re = 512 * (c // 2) + 128 * (c % 2)
        ro = re + 256
        r = res.results[c]["out"]
        out[0, re:re + 128, :] = r[0:128]
        out[1, re:re + 128, :] = r[128:256]
        out[0, ro:ro + 128, :] = r[256:384]
        out[1, ro:ro + 128, :] = r[384:512]
    return out
